# revision 1
# baseline (speedup 1.0000x reference)
"""RWKV-5 block (TimeMix + ChannelMix) on 8 Trainium2 NeuronCores.

Sharding: 2 batch groups x 4-way tensor-parallel (core = 4*g + lane).
TimeMix heads split 8/lane; (att*g)^T AllGathered per group, Wo replicated.
ChannelMix FF split 2048/lane; kv partials ReduceScattered by C rows.
Activations channel-major (x^T [C,T]); LN stats via PE ones-reduction;
WKV chunked (L=128) with precomputed decay power tables; matmuls float32r.
Host assembles the full [B,T,C] output from per-core row slices.
"""
import sys
import numpy as np

sys.path.insert(0, '/opt/trn_rl_repo')

B, T, C, H, N, FF = 2, 1024, 2048, 32, 64, 8192
EPS = 1e-5
L = 128            # WKV chunk length
NCH = T // L       # 8 chunks
NCORES = 8
LANES = 4
HPL = H // LANES   # 8 heads per lane
CHL = HPL * N      # 512 att channels per lane
FFL = FF // LANES  # 2048 ff channels per lane
KT = C // 128      # 16 contraction tiles
S = 512            # token free-dim chunk
GROUPS = [[0, 1, 2, 3], [4, 5, 6, 7]]

_PROGRAM = None


def _build_program():
    import concourse.bacc as bacc
    import concourse.tile as tile
    from concourse import mybir
    from contextlib import ExitStack

    F32 = mybir.dt.float32
    F32R = mybir.dt.float32r
    ALU = mybir.AluOpType
    ACT = mybir.ActivationFunctionType

    nc = bacc.Bacc("TRN2", target_bir_lowering=False, debug=False,
                   num_devices=NCORES)

    def din(name, shape):
        return nc.dram_tensor(name, shape, F32, kind="ExternalInput").ap()

    xT = din("xT", [C, T])
    Wr = din("Wr", [C, CHL]); Wk = din("Wk", [C, CHL])
    Wv = din("Wv", [C, CHL]); Wg = din("Wg", [C, CHL])
    Wo = din("Wo", [C, C])
    Wkey = din("Wkey", [C, FFL]); Wval = din("Wval", [FFL, C])
    Wrec = din("Wrec", [C, CHL])
    tmK = din("tmK", [C, 1]); tmV = din("tmV", [C, 1])
    tmR = din("tmR", [C, 1]); tmG = din("tmG", [C, 1])
    fmK = din("fmK", [C, 1]); fmR = din("fmR", [C, 1])
    POW_R = din("POW_R", [CHL, L]); POW_K = din("POW_K", [CHL, L])
    POW_U = din("POW_U", [CHL, L]); POW_CT = din("POW_CT", [L, CHL])
    DL = din("DL", [CHL, 1])
    MASKT = din("MASKT", [L, L]); IDENT = din("IDENT", [L, L])
    ONESC = din("ONESC", [128, 1]); ONESR = din("ONESR", [1, 128])
    ZERO64 = din("ZERO64", [128, 64])

    o1 = nc.dram_tensor("o1", [CHL, T], F32, kind="ExternalOutput").ap()
    x2out = nc.dram_tensor("x2out", [C, T], F32, kind="ExternalOutput").ap()

    cc_in = nc.dram_tensor("cc_in", [CHL, T], F32).ap()
    ag_out = nc.dram_tensor("ag_out", [C, T], F32).ap()
    rs_in = nc.dram_tensor("rs_in", [C, T], F32).ap()
    rs_out = nc.dram_tensor("rs_out", [CHL, T], F32).ap()
    kT_dram = nc.dram_tensor("kT_dram", [CHL, T], F32).ap()
    g_dram = nc.dram_tensor("g_dram", [T, CHL], F32).ap()
    ck_dram = nc.dram_tensor("ck_dram", [C, T], F32).ap()
    rT_dram = nc.dram_tensor("rT_dram", [CHL, T], F32).ap()

    with tile.TileContext(nc) as tc, ExitStack() as ctx:
        csts = ctx.enter_context(tc.tile_pool(name="csts", bufs=1))
        big = ctx.enter_context(tc.tile_pool(name="big", bufs=1))
        rot = ctx.enter_context(tc.tile_pool(name="rot", bufs=3))
        rot2 = ctx.enter_context(tc.tile_pool(name="rot2", bufs=2))
        outs = ctx.enter_context(tc.tile_pool(name="outs", bufs=1))
        wkvp = ctx.enter_context(tc.tile_pool(name="wkvp", bufs=3))
        state = ctx.enter_context(tc.tile_pool(name="state", bufs=2))
        ps_big = ctx.enter_context(
            tc.tile_pool(name="ps_big", bufs=4, space="PSUM"))
        ps_y = ctx.enter_context(tc.tile_pool(name="ps_y", bufs=1, space="PSUM"))
        ps_sd = ctx.enter_context(
            tc.tile_pool(name="ps_sd", bufs=1, space="PSUM"))
        ps_sm = ctx.enter_context(
            tc.tile_pool(name="ps_sm", bufs=2, space="PSUM"))

        # ---------------- constants ----------------
        _cst_n = [0]
        def load_const(ap, shape, rearr=None, dt=F32, p=128):
            _cst_n[0] += 1
            nm = f"cst{_cst_n[0]}"
            t = csts.tile(shape, dt, name=nm, tag=nm)
            src = ap if rearr is None else ap.rearrange(rearr, p=p)
            if dt == F32R:
                src = src.bitcast(F32R)
            nc.sync.dma_start(out=t, in_=src)
            return t

        tmK_t = load_const(tmK, [128, KT], "(kt p) o -> p (kt o)")
        tmV_t = load_const(tmV, [128, KT], "(kt p) o -> p (kt o)")
        tmR_t = load_const(tmR, [128, KT], "(kt p) o -> p (kt o)")
        tmG_t = load_const(tmG, [128, KT], "(kt p) o -> p (kt o)")
        fmK_t = load_const(fmK, [128, KT], "(kt p) o -> p (kt o)")
        fmR_t = load_const(fmR, [128, KT], "(kt p) o -> p (kt o)")
        powR_t = load_const(POW_R, [64, HPL, L], "(h p) i -> p h i", p=64)
        powK_t = load_const(POW_K, [64, HPL, L], "(h p) i -> p h i", p=64)
        powU_t = load_const(POW_U, [64, HPL, L], "(h p) i -> p h i", p=64)
        powCT_t = load_const(POW_CT, [128, CHL])
        dl_t = load_const(DL, [64, HPL], "(h p) o -> p (h o)", p=64)
        maskT_t = load_const(MASKT, [128, L])
        ident_t = load_const(IDENT, [128, L])
        ones_r = load_const(ONESC, [128, 1], dt=F32R)
        ones1_r = load_const(ONESR, [1, 128], dt=F32R)
        eps_t = csts.tile([1, 1], F32)
        nc.vector.memset(eps_t, EPS)
        geps_t = csts.tile([128, 1], F32)
        nc.vector.memset(geps_t, 64.0 * EPS)

        # ---------------- shared big slots ----------------
        def new_bigA():
            # 64KB/part: xn -> ag_sb -> xn2 -> kk
            return big.tile([128, KT, T], F32R, tag="bigA", name="bigA")

        def new_mid(nfloats):
            # 48KB/part: (rT|kc|vtok) then (srec|kvsb)
            return big.tile([128, nfloats], F32R, tag="mid", name="mid")

        def load_wslab(w_ap, col0, cols):
            # 32KB/part slot shared with amask
            t = big.tile([128, KT, cols], F32R, tag="wsl", name="wsl")
            nc.sync.dma_start(
                out=t, in_=w_ap[:, col0:col0 + cols].rearrange(
                    "(kt p) m -> p kt m", p=128).bitcast(F32R))
            return t

        # ---------------- helpers ----------------
        def ln_stats(get_tile):
            """get_tile(kt, fc) -> [128,S] F32R AP -> (m_bc, r_bc)."""
            m = outs.tile([1, T], F32R, tag="lnm", name="lnm")
            sums = outs.tile([1, T], F32, tag="lnsum", name="lnsum")
            sumsq = outs.tile([1, T], F32, tag="lnsumsq", name="lnsumsq")
            for fc in range(2):
                ps_s = ps_sm.tile([1, S], F32, tag="sm", name="pss")
                ps_q = ps_sm.tile([1, S], F32, tag="sm", name="psq")
                for kt in range(KT):
                    xt_ = get_tile(kt, fc)
                    sq = rot.tile([128, S], F32R, tag="r512f", name="sq")
                    nc.scalar.activation(out=sq, in_=xt_.bitcast(F32),
                                         func=ACT.Square)
                    nc.tensor.matmul(ps_s, ones_r, xt_,
                                     start=(kt == 0), stop=(kt == KT - 1))
                    nc.tensor.matmul(ps_q, ones_r, sq,
                                     start=(kt == 0), stop=(kt == KT - 1))
                nc.any.tensor_copy(out=sums[:, fc * S:(fc + 1) * S], in_=ps_s)
                nc.any.tensor_copy(out=sumsq[:, fc * S:(fc + 1) * S], in_=ps_q)
            nc.scalar.mul(out=m, in_=sums, mul=1.0 / C)
            tmp = outs.tile([1, T], F32, tag="lnsum", name="lntmp")
            nc.vector.tensor_mul(out=tmp, in0=m.bitcast(F32),
                                 in1=m.bitcast(F32))
            nc.scalar.mul(out=sumsq, in_=sumsq, mul=1.0 / C)
            nc.vector.tensor_sub(out=tmp, in0=sumsq, in1=tmp)
            nc.scalar.activation(out=tmp, in_=tmp, func=ACT.Sqrt, bias=eps_t)
            rstd = outs.tile([1, T], F32R, tag="lnrstd", name="lnrstd")
            with nc.allow_low_precision("f32r rstd for broadcast matmul"):
                nc.vector.reciprocal(out=rstd, in_=tmp)
            m_bc = outs.tile([128, 2, S], F32, tag="lnmbc", name="lnmbc")
            r_bc = outs.tile([128, 2, S], F32, tag="lnrbc", name="lnrbc")
            for fc in range(2):
                for vec, dst in ((m, m_bc), (rstd, r_bc)):
                    ps_b = ps_sm.tile([128, S], F32, tag="sm", name="psb")
                    nc.tensor.matmul(ps_b, ones1_r,
                                     vec[:, fc * S:(fc + 1) * S],
                                     start=True, stop=True)
                    nc.any.tensor_copy(out=dst[:, fc, :], in_=ps_b)
            return m_bc, r_bc

        def lerp_into(dst, xnbuf, tm_t, kt, fc):
            """dst [128,S] F32R AP <- time-lerp of xn tokens [fc*S,(fc+1)*S)."""
            sc = tm_t[:, kt:kt + 1]
            d = rot2.tile([128, S], F32, tag="dtile", name="dt")
            if fc == 0:
                nc.vector.tensor_sub(out=d[:, :S - 1],
                                     in0=xnbuf[:, kt, 1:S].bitcast(F32),
                                     in1=xnbuf[:, kt, 0:S - 1].bitcast(F32))
                nc.vector.scalar_tensor_tensor(
                    out=dst[:, 1:S], in0=d[:, :S - 1], scalar=sc,
                    in1=xnbuf[:, kt, 0:S - 1].bitcast(F32),
                    op0=ALU.mult, op1=ALU.add)
                nc.vector.tensor_scalar_mul(
                    out=dst[:, 0:1], in0=xnbuf[:, kt, 0:1].bitcast(F32),
                    scalar1=sc)
            else:
                nc.vector.tensor_sub(out=d,
                                     in0=xnbuf[:, kt, S:T].bitcast(F32),
                                     in1=xnbuf[:, kt, S - 1:T - 1].bitcast(F32))
                nc.vector.scalar_tensor_tensor(
                    out=dst, in0=d, scalar=sc,
                    in1=xnbuf[:, kt, S - 1:T - 1].bitcast(F32),
                    op0=ALU.mult, op1=ALU.add)

        def lerp_tile(xnbuf, tm_t, kt, fc):
            t = rot.tile([128, S], F32R, tag="r512f", name="lerp")
            lerp_into(t, xnbuf, tm_t, kt, fc)
            return t

        # ---------------- LN1 ----------------
        xn = new_bigA()
        nc.sync.dma_start(
            out=xn,
            in_=xT.rearrange("(kt p) t -> p kt t", p=128).bitcast(F32R))
        m_bc, r_bc = ln_stats(lambda kt, fc: xn[:, kt, fc * S:(fc + 1) * S])
        for kt in range(KT):
            for fc in range(2):
                sl = xn[:, kt, fc * S:(fc + 1) * S]
                slf = sl.bitcast(F32)
                nc.vector.tensor_sub(out=sl, in0=slf, in1=m_bc[:, fc, :])
                nc.vector.tensor_mul(out=sl, in0=slf, in1=r_bc[:, fc, :])

        # ---------------- TimeMix matmul phases ----------------
        mid = new_mid(8 * T)
        kc_v = mid[:, 0:4 * T].rearrange("p (c l) -> p c l", c=NCH)
        vtok_v = mid[:, 4 * T:8 * T].rearrange("p (c l) -> p c l", c=NCH)

        def ch_phase(w_t, tm_t, post):
            for fc in range(2):
                pss = [ps_big.tile([128, S], F32, tag="bm", name="pbm")
                       for _ in range(4)]
                for kt in range(KT):
                    rhs = lerp_tile(xn, tm_t, kt, fc)
                    for mt in range(4):
                        nc.tensor.matmul(
                            pss[mt], w_t[:, kt, mt * 128:(mt + 1) * 128], rhs,
                            start=(kt == 0), stop=(kt == KT - 1))
                for mt in range(4):
                    post(mt, fc, pss[mt])

        def tok_phase(w_t, tm_t, post):
            for half in range(2):
                pss = [ps_big.tile([128, CHL], F32, tag="bm", name="pbm")
                       for _ in range(4)]
                for kt in range(KT):
                    rhs = lerp_tile(xn, tm_t, kt, half)
                    for q in range(4):
                        nc.tensor.matmul(
                            pss[q], rhs[:, q * 128:(q + 1) * 128],
                            w_t[:, kt, :],
                            start=(kt == 0), stop=(kt == KT - 1))
                for q in range(4):
                    post(half * 4 + q, pss[q])

        wr_t = load_wslab(Wr, 0, CHL)
        def post_r(mt, fc, ps):
            rt_tile = rot.tile([128, S], F32, tag="r512", name="ro")
            nc.any.tensor_copy(out=rt_tile, in_=ps)
            nc.sync.dma_start(
                out=rT_dram[mt * 128:(mt + 1) * 128, fc * S:(fc + 1) * S],
                in_=rt_tile)
        ch_phase(wr_t, tmR_t, post_r)

        wk_t = load_wslab(Wk, 0, CHL)
        def post_k(mt, fc, ps):
            kt_tile = rot.tile([128, S], F32, tag="r512", name="ko")
            nc.any.tensor_copy(out=kt_tile, in_=ps)
            nc.sync.dma_start(
                out=kT_dram[mt * 128:(mt + 1) * 128, fc * S:(fc + 1) * S],
                in_=kt_tile)
        ch_phase(wk_t, tmK_t, post_k)

        def post_ktok(tt, ps):
            nc.vector.tensor_mul(out=kc_v[:, tt, :], in0=ps, in1=powCT_t)
        tok_phase(wk_t, tmK_t, post_ktok)

        wv_t = load_wslab(Wv, 0, CHL)
        def post_vtok(tt, ps):
            nc.any.tensor_copy(out=vtok_v[:, tt, :], in_=ps)
        tok_phase(wv_t, tmV_t, post_vtok)

        wg_t = load_wslab(Wg, 0, CHL)
        def post_gtok(tt, ps):
            gt = rot.tile([128, CHL], F32, tag="r512", name="go")
            nc.scalar.activation(out=gt, in_=ps, func=ACT.Silu)
            nc.sync.dma_start(out=g_dram[tt * 128:(tt + 1) * 128, :], in_=gt)
        tok_phase(wg_t, tmG_t, post_gtok)

        # ---------------- WKV pass 1: A^T, dv ----------------
        amask = big.tile([128, NCH, HPL, L], F32, tag="wsl", name="amask")
        dv_sb = outs.tile([128, NCH * HPL], F32, tag="dv", name="dv")
        for c in range(NCH):
            for h in range(HPL):
                kslab = wkvp.tile([64, L], F32, tag="kslab", name="ksl")
                nc.sync.dma_start(
                    out=kslab,
                    in_=kT_dram[h * 64:(h + 1) * 64, c * L:(c + 1) * L])
                rslab = wkvp.tile([64, L], F32, tag="rslab", name="rsl")
                nc.sync.dma_start(
                    out=rslab,
                    in_=rT_dram[h * 64:(h + 1) * 64, c * L:(c + 1) * L])
                rdT = wkvp.tile([64, L], F32R, tag="rdT", name="rdT")
                nc.vector.tensor_mul(out=rdT, in0=rslab,
                                     in1=powR_t[:, h, :])
                kdT = wkvp.tile([64, L], F32R, tag="kdT", name="kdT")
                nc.vector.tensor_mul(out=kdT, in0=kslab,
                                     in1=powK_t[:, h, :])
                kdU = wkvp.tile([64, L], F32R, tag="kdU", name="kdU")
                nc.vector.tensor_mul(out=kdU, in0=kslab,
                                     in1=powU_t[:, h, :])
                ps_a = ps_sm.tile([128, L], F32, tag="sm", name="psa")
                nc.tensor.matmul(ps_a, kdT, rdT, start=True, stop=True)
                nc.vector.tensor_mul(out=amask[:, c, h, :], in0=ps_a,
                                     in1=maskT_t)
                ps_b2 = ps_sm.tile([128, L], F32, tag="sm", name="psb2")
                nc.tensor.matmul(ps_b2, kdU, rdT, start=True, stop=True)
                bd = wkvp.tile([128, L], F32, tag="bd", name="bd", bufs=2)
                nc.vector.tensor_mul(out=bd, in0=ps_b2, in1=ident_t)
                with nc.allow_low_precision("dv diag sum"):
                    nc.vector.tensor_reduce(
                        out=dv_sb[:, c * 8 + h:c * 8 + h + 1], in_=bd,
                        axis=mybir.AxisListType.X, op=ALU.add)

        # ---------------- WKV pass 2 ----------------
        spairs = {}
        for h in range(HPL):
            sp = state.tile([64, 64], F32R, tag=f"St{h}", name="sp")
            nc.sync.dma_start(out=sp, in_=ZERO64[0:64, :].bitcast(F32R))
            spairs[h] = sp
        for c in range(NCH):
            gslab = wkvp.tile([128, CHL], F32, tag="gslab", name="gsl", bufs=2)
            nc.sync.dma_start(out=gslab, in_=g_dram[c * 128:(c + 1) * 128, :])
            attg_c = wkvp.tile([128, CHL], F32, tag="attgc", name="attgc", bufs=2)
            for h in range(HPL):
                rslab = wkvp.tile([64, L], F32, tag="rslab", name="rsl2")
                nc.sync.dma_start(
                    out=rslab,
                    in_=rT_dram[h * 64:(h + 1) * 64, c * L:(c + 1) * L])
                rdT = wkvp.tile([64, L], F32R, tag="rdT", name="rdT2")
                nc.vector.tensor_mul(out=rdT, in0=rslab,
                                     in1=powR_t[:, h, :])
                afin = wkvp.tile([128, L], F32R, tag="afin", name="afin")
                nc.vector.scalar_tensor_tensor(
                    out=afin, in0=ident_t,
                    scalar=dv_sb[:, c * 8 + h:c * 8 + h + 1],
                    in1=amask[:, c, h, :],
                    op0=ALU.mult, op1=ALU.add)
                S_pair = spairs[h]
                ps_yt = ps_y.tile([128, 64], F32, tag="yt", name="psy")
                nc.tensor.matmul(ps_yt, afin,
                                 vtok_v[:, c, h * 64:(h + 1) * 64],
                                 start=True, stop=False)
                nc.tensor.matmul(ps_yt, rdT, S_pair,
                                 start=False, stop=True)
                ps_d = ps_sd.tile([64, 64], F32, tag="sd", name="psd")
                nc.tensor.matmul(ps_d,
                                 kc_v[:, c, h * 64:(h + 1) * 64],
                                 vtok_v[:, c, h * 64:(h + 1) * 64],
                                 start=True, stop=True)
                S_new = state.tile([64, 64], F32R, tag=f"St{h}",
                                   name="snew")
                nc.vector.scalar_tensor_tensor(
                    out=S_new,
                    in0=S_pair.bitcast(F32),
                    scalar=dl_t[:, h:h + 1],
                    in1=ps_d,
                    op0=ALU.mult, op1=ALU.add)
                spairs[h] = S_new
                stats = wkvp.tile([128, 6], F32, tag="bnst", name="bnst")
                nc.vector.bn_stats(out=stats, in_=ps_yt)
                mv = wkvp.tile([128, 2], F32, tag="bnmv", name="bnmv")
                nc.vector.bn_aggr(out=mv, in_=stats)
                std = wkvp.tile([128, 1], F32, tag="bnstd", name="bnstd")
                nc.scalar.activation(out=std, in_=mv[:, 1:2],
                                     func=ACT.Sqrt, bias=geps_t)
                rstd = wkvp.tile([128, 1], F32, tag="bnrstd", name="bnr")
                nc.vector.reciprocal(out=rstd, in_=std)
                an = wkvp.tile([128, 64], F32, tag="an", name="an")
                nc.vector.tensor_scalar(
                    out=an, in0=ps_yt, scalar1=mv[:, 0:1], scalar2=rstd,
                    op0=ALU.subtract, op1=ALU.mult)
                nc.vector.tensor_mul(
                    out=attg_c[:, h * 64:(h + 1) * 64], in0=an,
                    in1=gslab[:, h * 64:(h + 1) * 64])
            for ct in range(4):
                ps_t = ps_sm.tile([128, L], F32, tag="sm", name="pst")
                nc.tensor.transpose(
                    ps_t, attg_c[:, ct * 128:(ct + 1) * 128], ident_t)
                tt_ = rot.tile([128, L], F32, tag="r512", name="tro")
                nc.any.tensor_copy(out=tt_, in_=ps_t)
                nc.sync.dma_start(
                    out=cc_in[ct * 128:(ct + 1) * 128, c * L:(c + 1) * L],
                    in_=tt_)

        nc.gpsimd.collective_compute(
            "AllGather", ALU.bypass, ins=[cc_in], outs=[ag_out],
            replica_groups=GROUPS)

        # ---------------- Wo + residual -> x2out ----------------
        ag_sb = new_bigA()
        nc.sync.dma_start(
            out=ag_sb,
            in_=ag_out.rearrange("(kt p) t -> p kt t", p=128).bitcast(F32R))
        for q in range(4):
            wo_t = load_wslab(Wo, q * S, S)
            for fc in range(2):
                pss = [ps_big.tile([128, S], F32, tag="bm", name="pbm")
                       for _ in range(4)]
                for kt in range(KT):
                    for mt in range(4):
                        nc.tensor.matmul(
                            pss[mt], wo_t[:, kt, mt * 128:(mt + 1) * 128],
                            ag_sb[:, kt, fc * S:(fc + 1) * S],
                            start=(kt == 0), stop=(kt == KT - 1))
                for mt in range(4):
                    gm = q * 4 + mt
                    xres = rot.tile([128, S], F32, tag="r512", name="xres")
                    nc.sync.dma_start(
                        out=xres,
                        in_=xT[gm * 128:(gm + 1) * 128, fc * S:(fc + 1) * S])
                    x2t = rot.tile([128, S], F32, tag="r512", name="x2t")
                    nc.vector.tensor_add(out=x2t, in0=pss[mt], in1=xres)
                    nc.sync.dma_start(
                        out=x2out[gm * 128:(gm + 1) * 128,
                                  fc * S:(fc + 1) * S],
                        in_=x2t)

        # ---------------- LN2 (stream x2out) -> xn2 ----------------
        def x2_tile(kt, fc):
            t = rot.tile([128, S], F32R, tag="r512f", name="x2l")
            nc.sync.dma_start(
                out=t, in_=x2out[kt * 128:(kt + 1) * 128,
                                 fc * S:(fc + 1) * S].bitcast(F32R))
            return t

        m2_bc, r2_bc = ln_stats(x2_tile)
        xn2 = new_bigA()
        for kt in range(KT):
            for fc in range(2):
                t = x2_tile(kt, fc)
                sl = xn2[:, kt, fc * S:(fc + 1) * S]
                nc.vector.tensor_sub(out=sl, in0=t.bitcast(F32),
                                     in1=m2_bc[:, fc, :])
                nc.vector.tensor_mul(out=sl, in0=sl.bitcast(F32),
                                     in1=r2_bc[:, fc, :])

        # ---------------- ChannelMix ----------------
        srk = new_mid(8 * T)
        srec = srk[:, 0:4 * T].rearrange("p (s t) -> p s t", s=4).bitcast(F32)
        kv_sb = srk[:, 4 * T:8 * T].rearrange("p (s t) -> p s t",
                                              s=4).bitcast(F32)
        wrec_t = load_wslab(Wrec, 0, CHL)
        for fc in range(2):
            pss = [ps_big.tile([128, S], F32, tag="bm", name="pbm")
                   for _ in range(4)]
            for kt in range(KT):
                rhs = lerp_tile(xn2, fmR_t, kt, fc)
                for mt in range(4):
                    nc.tensor.matmul(
                        pss[mt], wrec_t[:, kt, mt * 128:(mt + 1) * 128], rhs,
                        start=(kt == 0), stop=(kt == KT - 1))
            for mt in range(4):
                nc.scalar.activation(out=srec[:, mt, fc * S:(fc + 1) * S],
                                     in_=pss[mt], func=ACT.Sigmoid)

        for kt in range(KT):
            for fc in range(2):
                t = rot.tile([128, S], F32R, tag="r512f", name="cko")
                lerp_into(t, xn2, fmK_t, kt, fc)
                nc.sync.dma_start(
                    out=ck_dram[kt * 128:(kt + 1) * 128, fc * S:(fc + 1) * S],
                    in_=t.bitcast(F32))

        kk = new_bigA()
        for q in range(4):
            wkey_t = load_wslab(Wkey, q * S, S)
            for fc in range(2):
                pss = [ps_big.tile([128, S], F32, tag="bm", name="pbm")
                       for _ in range(4)]
                for kt in range(KT):
                    rhs = rot.tile([128, S], F32R, tag="r512f", name="ckl")
                    nc.sync.dma_start(
                        out=rhs,
                        in_=ck_dram[kt * 128:(kt + 1) * 128,
                                    fc * S:(fc + 1) * S].bitcast(F32R))
                    for mt in range(4):
                        nc.tensor.matmul(
                            pss[mt], wkey_t[:, kt, mt * 128:(mt + 1) * 128],
                            rhs, start=(kt == 0), stop=(kt == KT - 1))
                for mt in range(4):
                    rl = rot.tile([128, S], F32, tag="r512", name="rl")
                    nc.scalar.activation(out=rl, in_=pss[mt], func=ACT.Relu)
                    nc.vector.tensor_mul(
                        out=kk[:, q * 4 + mt, fc * S:(fc + 1) * S],
                        in0=rl, in1=rl)

        for q in range(4):
            wval_t = load_wslab(Wval, q * S, S)
            for fc in range(2):
                pss = [ps_big.tile([128, S], F32, tag="bm", name="pbm")
                       for _ in range(4)]
                for kt in range(KT):
                    for mt in range(4):
                        nc.tensor.matmul(
                            pss[mt], wval_t[:, kt, mt * 128:(mt + 1) * 128],
                            kk[:, kt, fc * S:(fc + 1) * S],
                            start=(kt == 0), stop=(kt == KT - 1))
                for mt in range(4):
                    kvt = rot.tile([128, S], F32, tag="r512", name="kvo")
                    nc.any.tensor_copy(out=kvt, in_=pss[mt])
                    gm = q * 4 + mt
                    nc.sync.dma_start(
                        out=rs_in[gm * 128:(gm + 1) * 128,
                                  fc * S:(fc + 1) * S],
                        in_=kvt)
        nc.gpsimd.collective_compute(
            "ReduceScatter", ALU.add, ins=[rs_in], outs=[rs_out],
            replica_groups=GROUPS)

        nc.sync.dma_start(
            out=kv_sb, in_=rs_out.rearrange("(mt p) t -> p mt t", p=128))
        for mt in range(4):
            for fc in range(2):
                ot = rot.tile([128, S], F32, tag="r512", name="ot")
                nc.vector.tensor_mul(out=ot,
                                     in0=srec[:, mt, fc * S:(fc + 1) * S],
                                     in1=kv_sb[:, mt, fc * S:(fc + 1) * S])
                nc.sync.dma_start(
                    out=o1[mt * 128:(mt + 1) * 128, fc * S:(fc + 1) * S],
                    in_=ot)

    nc.compile()
    return nc


def _host_inputs(inputs):
    f32 = np.float32
    x = np.asarray(inputs['x'], f32)
    for k in ('ln1_g', 'ln2_g', 'lnx_g'):
        assert np.allclose(np.asarray(inputs[k]), 1.0), f"{k} not identity"
    for k in ('ln1_b', 'ln2_b', 'lnx_b'):
        assert np.allclose(np.asarray(inputs[k]), 0.0), f"{k} not zero"

    dec = np.exp(-np.exp(np.asarray(inputs['time_decay'], np.float64)))
    u = np.asarray(inputs['time_faaaa'], np.float64)
    i_idx = np.arange(L, dtype=np.float64)

    maskT = np.tril(np.ones((L, L), f32), -1).T.copy()
    ident = np.eye(L, dtype=f32)

    def cvec(a):
        return np.ascontiguousarray(np.asarray(a, f32).reshape(C, 1))

    in_maps = []
    for core in range(NCORES):
        g, lane = divmod(core, LANES)
        hsl = slice(lane * HPL, (lane + 1) * HPL)
        dlh = dec[hsl]            # [HPL, N]
        ulh = u[hsl]
        pow_r = dlh[:, None, :] ** i_idx[None, :, None]            # [HPL,L,N]
        pow_k = dlh[:, None, :] ** (-(i_idx[None, :, None] + 1))
        pow_u = ulh[:, None, :] * dlh[:, None, :] ** (-i_idx[None, :, None])
        pow_c = dlh[:, None, :] ** (L - 1 - i_idx[None, :, None])

        def chmaj(p):   # [HPL, L, N] -> [CHL, L]
            return np.ascontiguousarray(
                p.transpose(0, 2, 1).reshape(CHL, L).astype(f32))

        POW_CT = np.ascontiguousarray(
            pow_c.transpose(1, 0, 2).reshape(L, CHL).astype(f32))
        csl = slice(lane * CHL, (lane + 1) * CHL)
        ffsl = slice(lane * FFL, (lane + 1) * FFL)
        in_maps.append({
            'xT': np.ascontiguousarray(x[g].T),
            'Wr': np.ascontiguousarray(np.asarray(inputs['Wr'], f32)[:, csl]),
            'Wk': np.ascontiguousarray(np.asarray(inputs['Wk'], f32)[:, csl]),
            'Wv': np.ascontiguousarray(np.asarray(inputs['Wv'], f32)[:, csl]),
            'Wg': np.ascontiguousarray(np.asarray(inputs['Wg'], f32)[:, csl]),
            'Wo': np.ascontiguousarray(np.asarray(inputs['Wo'], f32)),
            'Wkey': np.ascontiguousarray(
                np.asarray(inputs['Wkey'], f32)[:, ffsl]),
            'Wval': np.ascontiguousarray(
                np.asarray(inputs['Wval'], f32)[ffsl, :]),
            'Wrec': np.ascontiguousarray(np.asarray(inputs['Wrec'], f32)[:, csl]),
            'tmK': cvec(inputs['tm_k']), 'tmV': cvec(inputs['tm_v']),
            'tmR': cvec(inputs['tm_r']), 'tmG': cvec(inputs['tm_g']),
            'fmK': cvec(inputs['fm_k']), 'fmR': cvec(inputs['fm_r']),
            'POW_R': chmaj(pow_r), 'POW_K': chmaj(pow_k),
            'POW_U': chmaj(pow_u), 'POW_CT': POW_CT,
            'DL': np.ascontiguousarray((dlh ** L).reshape(CHL, 1).astype(f32)),
            'MASKT': maskT, 'IDENT': ident,
            'ONESC': np.ones((128, 1), f32),
            'ONESR': np.ones((1, 128), f32),
            'ZERO64': np.zeros((128, 64), f32),
        })
    return in_maps


_LAST_RESULT = {}


def kernel(**inputs):
    global _PROGRAM
    from concourse.bass_utils import run_bass_kernel_spmd
    if _PROGRAM is None:
        _PROGRAM = _build_program()
    in_maps = _host_inputs(inputs)
    trace = bool(int(__import__('os').environ.get('KERNEL_TRACE', '0')))
    res = run_bass_kernel_spmd(_PROGRAM, in_maps, list(range(NCORES)),
                               trace=trace)
    _LAST_RESULT['res'] = res
    out = np.empty((B, T, C), np.float32)
    for core in range(NCORES):
        g, lane = divmod(core, LANES)
        r = res.results[core]
        sl = slice(lane * CHL, (lane + 1) * CHL)
        out[g, :, sl] = (r['o1'] + r['x2out'][sl, :]).T
    return out



# revision 25
# speedup vs baseline: 1.4113x; 1.4113x over previous
"""RWKV-5 block (TimeMix + ChannelMix) on 8 Trainium2 NeuronCores.

Sharding: 2 batch groups x 4-way tensor-parallel (core = 4*g + lane).
TimeMix heads split 8/lane (processed as 4 head-pairs packed in 128
partitions); ChannelMix FF split 2048/lane.

v2 design vs baseline:
- bf16 operands for every matmul (full PE rate incl. small WKV matmuls,
  2x DVE rate, half DMA/SBUF), fp32 PSUM accumulation throughout.
- Channel-major WKV outputs (y = [ch, tok]) eliminate all transposes;
  GroupNorm stats via pair-mask matmuls.
- dv diagonal term computed as u*k*r (decay powers cancel at sigma=tau).
- All intermediates (r,k,kc,v,g,ck,kk,srec,xn,xn2) SBUF-resident; no
  DRAM roundtrips.
- AllGather of att*g chunked 4x and overlapped with the WKV recurrence;
  ReduceScatter of ChannelMix partials chunked 4x and overlapped with
  Wval matmuls.
- LN2 stats accumulated inline with Wo drains; time-shift handled by a
  zero-padded column (no boundary special case).
Host assembles the full [B,T,C] output from per-core row slices.
"""
import sys
import numpy as np

sys.path.insert(0, '/opt/trn_rl_repo')

B, T, C, H, N, FF = 2, 1024, 2048, 32, 64, 8192
EPS = 1e-5
L = 128            # WKV chunk length == token block
NCH = T // L       # 8 chunks
NCORES = 8
LANES = 4
HPL = H // LANES   # 8 heads per lane
NPAIR = HPL // 2   # 4 head pairs
CHL = HPL * N      # 512 att channels per lane
FFL = FF // LANES  # 2048 ff channels per lane
KT = C // 128      # 16 contraction tiles
S = 512            # token free-dim chunk
SP1 = T + 1        # padded token axis (col 0 = zeros for time-shift)
GROUPS = [[0, 1, 2, 3], [4, 5, 6, 7]]

_PROGRAM = None


def _build_program():
    import concourse.bacc as bacc
    import concourse.tile as tile
    from concourse import mybir
    from contextlib import ExitStack

    F32 = mybir.dt.float32
    F32R = mybir.dt.float32r
    BF16 = mybir.dt.bfloat16
    ALU = mybir.AluOpType
    ACT = mybir.ActivationFunctionType

    nc = bacc.Bacc("TRN2", target_bir_lowering=False, debug=False,
                   num_devices=NCORES)

    def din(name, shape, dt=F32):
        return nc.dram_tensor(name, shape, dt, kind="ExternalInput").ap()

    xT = din("xT", [C, T])
    Wr = din("Wr", [C, CHL], BF16); Wk = din("Wk", [C, CHL], BF16)
    Wv = din("Wv", [C, CHL], BF16); Wg = din("Wg", [C, CHL], BF16)
    Wo = din("Wo", [C, C], BF16)
    Wkey = din("Wkey", [C, FFL], BF16); Wval = din("Wval", [FFL, C], BF16)
    Wrec = din("Wrec", [C, CHL], BF16)
    TMK = din("TMK", [128, KT]); TMV = din("TMV", [128, KT])
    TMR = din("TMR", [128, KT]); TMG = din("TMG", [128, KT])
    FMK = din("FMK", [128, KT]); FMR = din("FMR", [128, KT])
    POWR = din("POWR", [128, NPAIR * L], BF16)
    POWK = din("POWK", [128, NPAIR * L], BF16)
    POWU0 = din("POWU0", [128, NPAIR])
    POWCT = din("POWCT", [128, CHL])
    DLT = din("DLT", [128, NPAIR])
    MASKT = din("MASKT", [128, L])
    IDENT = din("IDENT", [128, L], BF16)
    PAIRS = din("PAIRS", [128, 2], BF16)
    PAIRB = din("PAIRB", [2, 128])
    ONES1 = din("ONES1", [1, 128])
    ONESP = din("ONESP", [128, 1], BF16)

    o1 = nc.dram_tensor("o1", [CHL, T], F32, kind="ExternalOutput").ap()
    x2out = nc.dram_tensor("x2out", [C, T], F32, kind="ExternalOutput").ap()

    DEBUG = bool(int(__import__('os').environ.get('KERNEL_DEBUG', '0')))
    dbg_cc = (nc.dram_tensor("dbg_cc", [NCH * CHL, L], BF16,
                             kind="ExternalOutput").ap() if DEBUG else None)
    dbg_ag = (nc.dram_tensor("dbg_ag", [128, KT * NCH * L], BF16,
                             kind="ExternalOutput").ap() if DEBUG else None)
    dbg_rk = (nc.dram_tensor("dbg_rk", [128, 2 * NPAIR * T], BF16,
                             kind="ExternalOutput").ap() if DEBUG else None)
    dbg_vg = (nc.dram_tensor("dbg_vg", [128, NCH * CHL + NPAIR * T], BF16,
                             kind="ExternalOutput").ap() if DEBUG else None)
    dbg_kc = (nc.dram_tensor("dbg_kc", [128, NCH * CHL], BF16,
                             kind="ExternalOutput").ap() if DEBUG else None)
    cc_in = nc.dram_tensor("cc_in", [NCH * CHL, L], BF16).ap()
    ag_out = nc.dram_tensor("ag_out", [16, 2 * CHL * L], BF16).ap()
    rs_in = nc.dram_tensor("rs_in", [4 * C, 256], BF16).ap()
    rs_out = nc.dram_tensor("rs_out", [4 * (C // LANES), 256], BF16).ap()

    with tile.TileContext(nc) as tc, ExitStack() as ctx:
        csts = ctx.enter_context(tc.tile_pool(name="csts", bufs=1))
        big = ctx.enter_context(tc.tile_pool(name="big", bufs=1))
        wbuf = ctx.enter_context(tc.tile_pool(name="wbuf", bufs=1))
        slab = ctx.enter_context(tc.tile_pool(name="slab", bufs=2))
        rot = ctx.enter_context(tc.tile_pool(name="rot", bufs=2))
        rotb = ctx.enter_context(tc.tile_pool(name="rotb", bufs=2))
        wrot = ctx.enter_context(tc.tile_pool(name="wrot", bufs=2))
        state = ctx.enter_context(tc.tile_pool(name="state", bufs=2))
        ps_big = ctx.enter_context(
            tc.tile_pool(name="ps_big", bufs=4, space="PSUM"))
        ps_w1 = ctx.enter_context(
            tc.tile_pool(name="ps_w1", bufs=2, space="PSUM"))
        ps_w2 = ctx.enter_context(
            tc.tile_pool(name="ps_w2", bufs=2, space="PSUM"))

        # ---------------- constants ----------------
        _cst_n = [0]
        def load_const(ap, shape, dt=F32):
            _cst_n[0] += 1
            nm = f"cst{_cst_n[0]}"
            t = csts.tile(shape, dt, name=nm, tag=nm)
            src = ap
            if dt == F32R:
                src = src.bitcast(F32R)
            nc.sync.dma_start(out=t, in_=src)
            return t

        tmK_t = load_const(TMK, [128, KT])
        tmV_t = load_const(TMV, [128, KT])
        tmR_t = load_const(TMR, [128, KT])
        tmG_t = load_const(TMG, [128, KT])
        fmK_t = load_const(FMK, [128, KT])
        fmR_t = load_const(FMR, [128, KT])
        powR_t = load_const(POWR, [128, NPAIR * L], BF16)
        powR_t = powR_t.rearrange("p (m l) -> p m l", m=NPAIR)
        powK_t = load_const(POWK, [128, NPAIR * L], BF16)
        powK_t = powK_t.rearrange("p (m l) -> p m l", m=NPAIR)
        powU0_t = load_const(POWU0, [128, NPAIR])
        powCT_t = load_const(POWCT, [128, CHL])
        dl_t = load_const(DLT, [128, NPAIR])
        maskT_t = load_const(MASKT, [128, L])
        ident_t = load_const(IDENT, [128, L], BF16)
        pairs_t = load_const(PAIRS, [128, 2], BF16)
        pairb_r = load_const(PAIRB, [2, 128], F32R)
        ones1_r = load_const(ONES1, [1, 128], F32R)
        onesp_t = load_const(ONESP, [128, 1], BF16)
        eps_t = csts.tile([1, 1], F32)
        nc.vector.memset(eps_t, EPS)
        geps_t = csts.tile([2, 1], F32)
        nc.vector.memset(geps_t, float(N) * EPS)

        # ---------------- persistent SBUF buffers ----------------
        def bigslot(tag):
            # 32.8KB/partition slots, reused across phases via tags
            return big.tile([128, KT, SP1], BF16, tag=tag, name=tag)

        xn = bigslot("bigA")       # LN1(x), padded; later: ck
        lerpK = bigslot("bigB")    # k-lerp cache; later: xn2 (padded)
        agk = big.tile([128, KT, NCH, L], BF16, tag="bigC", name="agk")
        # agk: AllGather result [p, kt, c, l]; later reused for kk via tag

        rT = wbuf.tile([128, NPAIR, T], BF16, tag="rT", name="rT")
        kT = wbuf.tile([128, NPAIR, T], BF16, tag="kT", name="kT")
        kc = wbuf.tile([128, NCH, CHL], BF16, tag="kc", name="kc")
        vtok = wbuf.tile([128, NCH, CHL], BF16, tag="vtok", name="vtok")
        gch = wbuf.tile([128, NPAIR, T], BF16, tag="gch", name="gch")
        mb_ln1 = wbuf.tile([128, 2, S], BF16, tag="mb1", name="mb1")
        rb_ln1 = wbuf.tile([128, 2, S], BF16, tag="rb1", name="rb1")

        nc.vector.memset(xn[:, :, 0:1], 0.0)
        nc.vector.memset(lerpK[:, :, 0:1], 0.0)

        def load_slab(w_ap, col0):
            # two half-slabs of 256 cols each (8KB/part, bufs=2 rotation)
            halves = []
            for h in range(2):
                t = slab.tile([128, KT, 256], BF16, tag="wsl", name="wsl")
                nc.sync.dma_start(
                    out=t, in_=w_ap[:, col0 + h * 256:
                                    col0 + (h + 1) * 256].rearrange(
                        "(kt p) m -> p kt m", p=128))
                halves.append(t)
            return halves

        def wslice(w_t, kt, mt):
            return w_t[mt // 2][:, kt, (mt % 2) * 128:(mt % 2 + 1) * 128]

        # ================ LN1: load x, stats, normalize ================
        ln1_s = [None, None]
        ln1_q = [None, None]
        for fc in range(2):
            s_t = ps_w1.tile([128, S], F32, tag="w1", name="lnps")
            q_t = ps_w2.tile([128, S], F32, tag="w2", name="lnpq")
            ln1_s[fc] = s_t
            ln1_q[fc] = q_t
            for kt in range(KT):
                xf = rot.tile([128, S], F32, tag="xf", name="xf")
                nc.sync.dma_start(
                    out=xf, in_=xT[kt * 128:(kt + 1) * 128,
                                   fc * S:(fc + 1) * S])
                dst = xn[:, kt, 1 + fc * S:1 + (fc + 1) * S]
                nc.gpsimd.tensor_copy(out=dst, in_=xf)
                sq = rotb.tile([128, S], BF16, tag="sqb", name="sqb")
                nc.vector.tensor_mul(out=sq, in0=dst, in1=dst)
                nc.tensor.matmul(s_t[0:1, :], onesp_t, dst,
                                 start=(kt == 0), stop=(kt == KT - 1))
                nc.tensor.matmul(q_t[0:1, :], onesp_t, sq,
                                 start=(kt == 0), stop=(kt == KT - 1))

        def ln_finalize(s_ts, q_ts, mb, rb):
            for fc in range(2):
                m = rot.tile([1, S], F32R, tag="lnm", name="lnm", bufs=1)
                t1 = rot.tile([1, S], F32, tag="lnt1", name="lnt1", bufs=1)
                t2 = rot.tile([1, S], F32, tag="lnt2", name="lnt2", bufs=1)
                rstd = rot.tile([1, S], F32R, tag="lnr", name="lnr", bufs=1)
                with nc.allow_low_precision("f32r LN vecs for broadcast mm"):
                    nc.vector.tensor_scalar_mul(out=m, in0=s_ts[fc][0:1, :],
                                                scalar1=1.0 / C)
                nc.vector.tensor_scalar_mul(out=t1, in0=q_ts[fc][0:1, :],
                                            scalar1=1.0 / C)
                nc.vector.tensor_mul(out=t2, in0=m.bitcast(F32),
                                     in1=m.bitcast(F32))
                nc.vector.tensor_sub(out=t1, in0=t1, in1=t2)
                nc.scalar.activation(out=t2, in_=t1,
                                     func=ACT.Sqrt, bias=eps_t)
                with nc.allow_low_precision("f32r LN vecs for broadcast mm"):
                    nc.vector.reciprocal(out=rstd, in_=t2)
                for vsl, dst in ((m, mb), (rstd, rb)):
                    ps_b = ps_big.tile([128, S], F32, tag="bm", name="psb")
                    nc.tensor.matmul(ps_b, ones1_r, vsl,
                                     start=True, stop=True)
                    nc.scalar.copy(out=dst[:, fc, :], in_=ps_b)

        ln_finalize(ln1_s, ln1_q, mb_ln1, rb_ln1)
        for fc in range(2):
            for kt in range(KT):
                sl = xn[:, kt, 1 + fc * S:1 + (fc + 1) * S]
                nc.vector.tensor_sub(out=sl, in0=sl, in1=mb_ln1[:, fc, :])
                nc.vector.tensor_mul(out=sl, in0=sl, in1=rb_ln1[:, fc, :])

        # ================ projections ================
        def lerp_into(dst, src, tm_t, kt, c0, cols):
            """dst <- shift-lerp of src tokens [c0, c0+cols) (padded buf)."""
            xl = src[:, kt, 1 + c0:1 + c0 + cols]
            xx = src[:, kt, c0:c0 + cols]
            d = rotb.tile([128, S], BF16, tag="lerpd", name="lerpd")
            nc.vector.tensor_sub(out=d[:, :cols], in0=xl, in1=xx)
            nc.vector.scalar_tensor_tensor(
                out=dst, in0=d[:, :cols], scalar=tm_t[:, kt:kt + 1],
                in1=xx, op0=ALU.mult, op1=ALU.add)

        def lerp_tile(src, tm_t, kt, fc):
            t = rotb.tile([128, S], BF16, tag="lerpt", name="lerpt")
            lerp_into(t, src, tm_t, kt, fc * S, S)
            return t

        def ch_phase(w_t, get_rhs, post):
            for fc in range(2):
                pss = [ps_big.tile([128, S], F32, tag="bm", name="pbm")
                       for _ in range(4)]
                for kt in range(KT):
                    rhs = get_rhs(kt, fc)
                    for mt in range(4):
                        nc.tensor.matmul(
                            pss[mt], wslice(w_t, kt, mt),
                            rhs, start=(kt == 0), stop=(kt == KT - 1))
                for mt in range(4):
                    post(mt, fc, pss[mt])

        # P1: r (channel-major)
        wr_t = load_slab(Wr, 0)
        def post_r(mt, fc, ps):
            nc.scalar.copy(out=rT[:, mt, fc * S:(fc + 1) * S], in_=ps)
        ch_phase(wr_t, lambda kt, fc: lerp_tile(xn, tmR_t, kt, fc), post_r)

        # P2: k channel-major; k-lerp cached into lerpK
        wk_t = load_slab(Wk, 0)
        def krhs(kt, fc):
            dst = lerpK[:, kt, 1 + fc * S:1 + (fc + 1) * S]
            lerp_into(dst, xn, tmK_t, kt, fc * S, S)
            return dst
        def post_k(mt, fc, ps):
            nc.scalar.copy(out=kT[:, mt, fc * S:(fc + 1) * S], in_=ps)
        ch_phase(wk_t, krhs, post_k)

        # P3: k token-major -> kc (uses cached lerpK, same Wk slab)
        def tok_phase(w_t, get_lhs, post):
            # each (q, hf) accumulation chain owns a distinct PSUM bank:
            # interleaved chains within one bank corrupt each other
            # (start=True wipes the whole bank).
            for half in range(2):
                pss = {}
                for q in range(4):
                    pss[(q, 0)] = ps_big.tile([128, 256], F32, tag="bm",
                                              name="pbm")
                for q in range(2):
                    pss[(q, 1)] = ps_w1.tile([128, 256], F32, tag="w1",
                                             name="pw1")
                for q in range(2, 4):
                    pss[(q, 1)] = ps_w2.tile([128, 256], F32, tag="w2",
                                             name="pw2")
                for kt in range(KT):
                    lhs = get_lhs(kt, half)
                    for q in range(4):
                        for hf in range(2):
                            nc.tensor.matmul(
                                pss[(q, hf)],
                                lhs[:, q * 128:(q + 1) * 128],
                                w_t[hf][:, kt, :],
                                start=(kt == 0), stop=(kt == KT - 1))
                for q in range(4):
                    post(half * 4 + q, pss[(q, 0)], pss[(q, 1)])

        def post_kc(tb, ps0, ps1):
            nc.vector.tensor_mul(out=kc[:, tb, 0:256], in0=ps0,
                                 in1=powCT_t[:, 0:256])
            nc.vector.tensor_mul(out=kc[:, tb, 256:512], in0=ps1,
                                 in1=powCT_t[:, 256:512])
        tok_phase(wk_t,
                  lambda kt, half: lerpK[:, kt, 1 + half * S:1 + (half + 1) * S],
                  post_kc)

        # P4: v token-major
        wv_t = load_slab(Wv, 0)
        def post_v(tb, ps0, ps1):
            nc.scalar.copy(out=vtok[:, tb, 0:256], in_=ps0)
            nc.scalar.copy(out=vtok[:, tb, 256:512], in_=ps1)
        tok_phase(wv_t,
                  lambda kt, half: lerp_tile(xn, tmV_t, kt, half),
                  post_v)

        # P5: g channel-major + SiLU
        wg_t = load_slab(Wg, 0)
        def post_g(mt, fc, ps):
            nc.scalar.activation(out=gch[:, mt, fc * S:(fc + 1) * S],
                                 in_=ps, func=ACT.Silu)
        ch_phase(wg_t, lambda kt, fc: lerp_tile(xn, tmG_t, kt, fc), post_g)

        if DEBUG:
            nc.sync.dma_start(out=dbg_rk[:, 0:NPAIR * T],
                              in_=rT.rearrange("p m t -> p (m t)"))
            nc.sync.dma_start(out=dbg_rk[:, NPAIR * T:],
                              in_=kT.rearrange("p m t -> p (m t)"))
            nc.sync.dma_start(out=dbg_vg[:, 0:NCH * CHL],
                              in_=vtok.rearrange("p c n -> p (c n)"))
            nc.sync.dma_start(out=dbg_vg[:, NCH * CHL:],
                              in_=gch.rearrange("p m t -> p (m t)"))
            nc.sync.dma_start(out=dbg_kc,
                              in_=kc.rearrange("p c n -> p (c n)"))

        # ================ WKV recurrence (head pairs) ================
        spairs = {}
        for m in range(NPAIR):
            sp = state.tile([128, 64], BF16, tag=f"St{m}", name="sp")
            nc.vector.memset(sp, 0.0)
            spairs[m] = sp

        for c in range(NCH):
            csl = slice(c * L, (c + 1) * L)
            for m in range(NPAIR):
                rsl = rT[:, m, csl]
                ksl = kT[:, m, csl]
                rdT = wrot.tile([128, L], BF16, tag="rdT", name="rdT")
                nc.vector.tensor_mul(out=rdT, in0=rsl, in1=powR_t[:, m, :])
                kdT = wrot.tile([128, L], BF16, tag="kdT", name="kdT")
                nc.vector.tensor_mul(out=kdT, in0=ksl, in1=powK_t[:, m, :])
                prod = wrot.tile([128, L], BF16, tag="prod", name="prod")
                nc.vector.tensor_mul(out=prod, in0=rsl, in1=ksl)
                nc.vector.tensor_scalar_mul(out=prod, in0=prod,
                                            scalar1=powU0_t[:, m:m + 1])
                # one PSUM bank per accumulation chain (interleaved chains
                # within a bank corrupt each other)
                psA = [ps_w1.tile([128, L], F32, tag="w1", name="psa")
                       for _ in range(2)]
                psDV = [ps_w2.tile([128, 1], F32, tag="w2", name="psdv")
                        for _ in range(2)]
                psY = [ps_big.tile([128, L], F32, tag="bm", name="psy")
                       for _ in range(2)]
                psD = [ps_big.tile([128, 64], F32, tag="bm", name="psd")
                       for _ in range(2)]
                afins = []
                for hh in range(2):
                    b = hh * 64
                    nc.tensor.matmul(psA[hh], kdT[b:b + 64, :],
                                     rdT[b:b + 64, :], start=True, stop=True)
                    nc.tensor.matmul(psDV[hh], prod[b:b + 64, :],
                                     onesp_t[b:b + 64, :],
                                     start=True, stop=True)
                    am = wrot.tile([128, L], BF16, tag="am", name="am")
                    nc.vector.tensor_mul(out=am, in0=psA[hh], in1=maskT_t)
                    dvb = wrot.tile([128, 1], F32, tag="dvb", name="dvb")
                    nc.vector.tensor_copy(out=dvb, in_=psDV[hh])
                    afin = wrot.tile([128, L], BF16, tag="afin", name="afin")
                    nc.vector.scalar_tensor_tensor(
                        out=afin, in0=ident_t, scalar=dvb, in1=am,
                        op0=ALU.mult, op1=ALU.add)
                    afins.append(afin)
                S_pair = spairs[m]
                for hh in range(2):
                    b = hh * 64
                    head = 2 * m + hh
                    hsl = slice(head * 64, (head + 1) * 64)
                    nc.tensor.matmul(psY[hh][b:b + 64, :],
                                     vtok[:, c, hsl], afins[hh],
                                     start=True, stop=False)
                    nc.tensor.matmul(psY[hh][b:b + 64, :],
                                     S_pair[b:b + 64, :], rdT[b:b + 64, :],
                                     start=False, stop=True)
                    nc.tensor.matmul(psD[hh][b:b + 64, :],
                                     kc[:, c, hsl], vtok[:, c, hsl],
                                     start=True, stop=True)
                psDb = wrot.tile([128, 64], BF16, tag="psDb", name="psDb")
                nc.scalar.copy(out=psDb[0:64, :], in_=psD[0][0:64, :])
                nc.scalar.copy(out=psDb[64:128, :], in_=psD[1][64:128, :])
                S_new = state.tile([128, 64], BF16, tag=f"St{m}", name="snew")
                nc.vector.scalar_tensor_tensor(
                    out=S_new, in0=S_pair, scalar=dl_t[:, m:m + 1],
                    in1=psDb, op0=ALU.mult, op1=ALU.add)
                spairs[m] = S_new

                # GroupNorm over each head's 64 channels + gate
                y_sb = wrot.tile([128, L], BF16, tag="ysb", name="ysb")
                nc.scalar.copy(out=y_sb[0:64, :], in_=psY[0][0:64, :])
                nc.scalar.copy(out=y_sb[64:128, :], in_=psY[1][64:128, :])
                ysq = wrot.tile([128, L], BF16, tag="ysq", name="ysq")
                nc.vector.tensor_mul(out=ysq, in0=y_sb, in1=y_sb)
                pstS = ps_w1.tile([2, L], F32, tag="w1", name="pstS")
                pstQ = ps_w1.tile([2, L], F32, tag="w1", name="pstQ")
                nc.tensor.matmul(pstS, pairs_t, y_sb,
                                 start=True, stop=True)
                nc.tensor.matmul(pstQ, pairs_t, ysq,
                                 start=True, stop=True)
                gn = wrot.tile([2, 2 * L], F32R, tag="gn", name="gn", bufs=1)
                m_sb = gn[:, 0:L]
                rstd = gn[:, L:2 * L]
                gn2 = wrot.tile([2, 2 * L], F32, tag="gn2", name="gn2", bufs=1)
                with nc.allow_low_precision("f32r GN vecs for broadcast mm"):
                    nc.vector.tensor_scalar_mul(out=m_sb, in0=pstS,
                                                scalar1=1.0 / N)
                nc.vector.tensor_scalar_mul(out=gn2[:, 0:L],
                                            in0=pstQ,
                                            scalar1=1.0 / N)
                nc.vector.tensor_mul(out=gn2[:, L:2 * L],
                                     in0=m_sb.bitcast(F32),
                                     in1=m_sb.bitcast(F32))
                nc.vector.tensor_sub(out=gn2[:, 0:L],
                                     in0=gn2[:, 0:L], in1=gn2[:, L:2 * L])
                gstd = wrot.tile([2, L], F32, tag="gstd", name="gstd", bufs=1)
                nc.scalar.activation(out=gstd, in_=gn2[:, 0:L],
                                     func=ACT.Sqrt, bias=geps_t)
                with nc.allow_low_precision("f32r GN vecs for broadcast mm"):
                    nc.vector.reciprocal(out=rstd, in_=gstd)
                psM = ps_w2.tile([128, L], F32, tag="w2", name="psM")
                psR = ps_w2.tile([128, L], F32, tag="w2", name="psR")
                nc.tensor.matmul(psM, pairb_r, m_sb, start=True, stop=True)
                nc.tensor.matmul(psR, pairb_r, rstd, start=True, stop=True)
                mbc = wrot.tile([128, L], BF16, tag="mbc", name="mbc")
                nc.scalar.copy(out=mbc, in_=psM)
                rbc = wrot.tile([128, L], BF16, tag="rbc", name="rbc")
                nc.scalar.copy(out=rbc, in_=psR)
                an = wrot.tile([128, L], BF16, tag="an", name="an")
                nc.vector.tensor_sub(out=an, in0=y_sb, in1=mbc)
                nc.vector.tensor_mul(out=an, in0=an, in1=rbc)
                attg = wrot.tile([128, L], BF16, tag="attg", name="attg")
                nc.vector.tensor_mul(out=attg, in0=an, in1=gch[:, m, csl])
                nc.sync.dma_start(
                    out=cc_in[c * CHL + m * 128:c * CHL + (m + 1) * 128, :],
                    in_=attg)
                if DEBUG:
                    nc.sync.dma_start(
                        out=dbg_cc[c * CHL + m * 128:
                                   c * CHL + (m + 1) * 128, :],
                        in_=attg)
            if c % 2 == 1:
                j = c // 2
                nc.gpsimd.collective_compute(
                    "AllGather", ALU.bypass,
                    ins=[cc_in[(c - 1) * CHL:(c + 1) * CHL, :]],
                    outs=[ag_out[j * 4:(j + 1) * 4, :]],
                    replica_groups=GROUPS)
                half_n = CHL * L
                for r in range(4):
                    for ci in range(2):
                        nc.sync.dma_start(
                            out=agk[:, r * 4:(r + 1) * 4, c - 1 + ci, :],
                            in_=ag_out[j * 4 + r:j * 4 + r + 1,
                                       ci * half_n:(ci + 1) * half_n
                                       ].rearrange(
                                "o (ktl p l) -> p (o ktl) l",
                                ktl=4, p=128))

        # ================ Wo + residual + LN2 stats inline ================
        xn2 = lerpK  # reuse bigB slot (k-lerp cache dead)
        ln2_s = [ps_w1.tile([128, S], F32, tag="w1", name="l2s0"),
                 ps_w1.tile([128, S], F32, tag="w1", name="l2s1")]
        ln2_q = [ps_w2.tile([128, S], F32, tag="w2", name="l2q0"),
                 ps_w2.tile([128, S], F32, tag="w2", name="l2q1")]
        for q in range(4):
            wo_t = load_slab(Wo, q * S)
            for fc in range(2):
                pss = [ps_big.tile([128, S], F32, tag="bm", name="pbm")
                       for _ in range(4)]
                for kt in range(KT):
                    rhs = agk[:, kt, fc * 4:(fc + 1) * 4, :]
                    for mt in range(4):
                        nc.tensor.matmul(
                            pss[mt], wslice(wo_t, kt, mt),
                            rhs, start=(kt == 0), stop=(kt == KT - 1))
                for mt in range(4):
                    gm = q * 4 + mt
                    xres = rot.tile([128, S], F32, tag="xf", name="xres")
                    nc.sync.dma_start(
                        out=xres,
                        in_=xT[gm * 128:(gm + 1) * 128, fc * S:(fc + 1) * S])
                    x2t = rot.tile([128, S], F32, tag="x2t", name="x2t")
                    nc.vector.tensor_add(out=x2t, in0=pss[mt], in1=xres)
                    nc.sync.dma_start(
                        out=x2out[gm * 128:(gm + 1) * 128,
                                  fc * S:(fc + 1) * S],
                        in_=x2t)
                    dst = xn2[:, gm, 1 + fc * S:1 + (fc + 1) * S]
                    nc.gpsimd.tensor_copy(out=dst, in_=x2t)
                    sq2 = rotb.tile([128, S], BF16, tag="sqb", name="sq2")
                    nc.vector.tensor_mul(out=sq2, in0=dst, in1=dst)
                    first = (q == 0 and mt == 0)
                    last = (q == 3 and mt == 3)
                    nc.tensor.matmul(ln2_s[fc][0:1, :], onesp_t, dst,
                                     start=first, stop=last)
                    nc.tensor.matmul(ln2_q[fc][0:1, :], onesp_t, sq2,
                                     start=first, stop=last)

        if DEBUG:
            nc.sync.dma_start(
                out=dbg_ag,
                in_=agk.rearrange("p kt c l -> p (kt c l)"))
        mb_ln2 = wbuf.tile([128, 2, S], BF16, tag="mb1", name="mb2")
        rb_ln2 = wbuf.tile([128, 2, S], BF16, tag="rb1", name="rb2")
        ln_finalize(ln2_s, ln2_q, mb_ln2, rb_ln2)
        for fc in range(2):
            for kt in range(KT):
                sl = xn2[:, kt, 1 + fc * S:1 + (fc + 1) * S]
                nc.vector.tensor_sub(out=sl, in0=sl, in1=mb_ln2[:, fc, :])
                nc.vector.tensor_mul(out=sl, in0=sl, in1=rb_ln2[:, fc, :])

        # ================ ChannelMix ================
        ck = xn  # reuse bigA slot (xn dead)
        srec = wbuf.tile([128, 4, T], BF16, tag="rT", name="srec")
        wrec_t = load_slab(Wrec, 0)
        for fc in range(2):
            pss = [ps_big.tile([128, S], F32, tag="bm", name="pbm")
                   for _ in range(4)]
            for kt in range(KT):
                rhs = lerp_tile(xn2, fmR_t, kt, fc)
                lerp_into(ck[:, kt, 1 + fc * S:1 + (fc + 1) * S],
                          xn2, fmK_t, kt, fc * S, S)
                for mt in range(4):
                    nc.tensor.matmul(
                        pss[mt], wslice(wrec_t, kt, mt),
                        rhs, start=(kt == 0), stop=(kt == KT - 1))
            for mt in range(4):
                nc.scalar.activation(out=srec[:, mt, fc * S:(fc + 1) * S],
                                     in_=pss[mt], func=ACT.Sigmoid)

        kk = agk.rearrange("p kt c l -> p kt (c l)")  # reuse bigC (ag dead)
        for q in range(4):
            wkey_t = load_slab(Wkey, q * S)
            for fc in range(2):
                pss = [ps_big.tile([128, S], F32, tag="bm", name="pbm")
                       for _ in range(4)]
                for kt in range(KT):
                    for mt in range(4):
                        nc.tensor.matmul(
                            pss[mt], wslice(wkey_t, kt, mt),
                            ck[:, kt, 1 + fc * S:1 + (fc + 1) * S],
                            start=(kt == 0), stop=(kt == KT - 1))
                for mt in range(4):
                    rl = rotb.tile([128, S], BF16, tag="rlb", name="rl")
                    nc.vector.tensor_scalar_max(out=rl, in0=pss[mt],
                                                scalar1=0.0)
                    nc.vector.tensor_mul(
                        out=kk[:, q * 4 + mt, fc * S:(fc + 1) * S],
                        in0=rl, in1=rl)

        for fc in range(2):
            for q in range(4):
                wval_t = load_slab(Wval, q * S)
                pss = [ps_big.tile([128, S], F32, tag="bm", name="pbm")
                       for _ in range(4)]
                for kt in range(KT):
                    for mt in range(4):
                        nc.tensor.matmul(
                            pss[mt], wslice(wval_t, kt, mt),
                            kk[:, kt, fc * S:(fc + 1) * S],
                            start=(kt == 0), stop=(kt == KT - 1))
                for mt in range(4):
                    gm = q * 4 + mt
                    kvt = rotb.tile([128, S], BF16, tag="kvt", name="kvt")
                    nc.scalar.copy(out=kvt, in_=pss[mt])
                    for h in range(2):
                        ch = fc * 2 + h
                        nc.sync.dma_start(
                            out=rs_in[ch * C + gm * 128:
                                      ch * C + (gm + 1) * 128, :],
                            in_=kvt[:, h * 256:(h + 1) * 256])
            for h in range(2):
                ch = fc * 2 + h
                nc.gpsimd.collective_compute(
                    "ReduceScatter", ALU.add,
                    ins=[rs_in[ch * C:(ch + 1) * C, :]],
                    outs=[rs_out[ch * CHL:(ch + 1) * CHL, :]],
                    replica_groups=GROUPS)

        for ch in range(4):
            kvsb = rotb.tile([128, 4, 256], BF16, tag="kvsb", name="kvsb", bufs=1)
            nc.sync.dma_start(
                out=kvsb,
                in_=rs_out[ch * CHL:(ch + 1) * CHL, :].rearrange(
                    "(blk p) t -> p blk t", p=128))
            for blk in range(4):
                ot = rot.tile([128, 256], F32, tag="ot", name="ot", bufs=1)
                nc.vector.tensor_mul(
                    out=ot, in0=srec[:, blk, ch * 256:(ch + 1) * 256],
                    in1=kvsb[:, blk, :])
                nc.sync.dma_start(
                    out=o1[blk * 128:(blk + 1) * 128,
                           ch * 256:(ch + 1) * 256],
                    in_=ot)

    nc.compile()
    return nc


def _host_inputs(inputs):
    import ml_dtypes
    f32 = np.float32
    bf16 = ml_dtypes.bfloat16
    x = np.asarray(inputs['x'], f32)
    for k in ('ln1_g', 'ln2_g', 'lnx_g'):
        assert np.allclose(np.asarray(inputs[k]), 1.0), f"{k} not identity"
    for k in ('ln1_b', 'ln2_b', 'lnx_b'):
        assert np.allclose(np.asarray(inputs[k]), 0.0), f"{k} not zero"

    dec = np.exp(-np.exp(np.asarray(inputs['time_decay'], np.float64)))
    u = np.asarray(inputs['time_faaaa'], np.float64)
    tau = np.arange(L, dtype=np.float64)

    maskT = np.tril(np.ones((L, L), f32), -1).T.copy()  # [sigma, tau]
    ident = np.eye(L, dtype=bf16)

    def chvec(a, name):
        # [C] -> [128, KT] with channel c = kt*128 + p
        v = np.asarray(a, f32).reshape(C)
        return np.ascontiguousarray(v.reshape(KT, 128).T)

    pairs = np.zeros((128, 2), bf16)
    pairs[0:64, 0] = 1
    pairs[64:128, 1] = 1
    pairb = np.zeros((2, 128), f32)
    pairb[0, 0:64] = 1
    pairb[1, 64:128] = 1

    in_maps = []
    for core in range(NCORES):
        g, lane = divmod(core, LANES)
        hsl = slice(lane * HPL, (lane + 1) * HPL)
        dlh = dec[hsl]            # [HPL, N]
        ulh = u[hsl]

        # pair layout value[p, m] from per-head [HPL, N]:
        # head = 2m + p//64, n = p%64
        def pair2(a):  # [HPL, N] -> [128, NPAIR]
            out = np.empty((128, NPAIR), np.float64)
            for m in range(NPAIR):
                out[0:64, m] = a[2 * m]
                out[64:128, m] = a[2 * m + 1]
            return out

        def pair3(a):  # [HPL, N, L] -> [128, NPAIR*L]
            out = np.empty((128, NPAIR, L), np.float64)
            for m in range(NPAIR):
                out[0:64, m, :] = a[2 * m]
                out[64:128, m, :] = a[2 * m + 1]
            return out.reshape(128, NPAIR * L)

        pow_r = dlh[:, :, None] ** tau[None, None, :]          # [HPL,N,L]
        pow_k = dlh[:, :, None] ** (-(tau[None, None, :] + 1))
        # POWCT: [sigma, ch] = d_ch^(L-1-sigma), ch = hl*64 + n
        pow_c = (dlh[:, :, None] ** (L - 1 - tau[None, None, :]))
        POW_CT = np.ascontiguousarray(
            pow_c.transpose(2, 0, 1).reshape(L, CHL).astype(f32))

        csl = slice(lane * CHL, (lane + 1) * CHL)
        ffsl = slice(lane * FFL, (lane + 1) * FFL)
        wbf = lambda a: np.ascontiguousarray(np.asarray(a, f32)).astype(bf16)
        in_maps.append({
            'xT': np.ascontiguousarray(x[g].T),
            'Wr': wbf(np.asarray(inputs['Wr'], f32)[:, csl]),
            'Wk': wbf(np.asarray(inputs['Wk'], f32)[:, csl]),
            'Wv': wbf(np.asarray(inputs['Wv'], f32)[:, csl]),
            'Wg': wbf(np.asarray(inputs['Wg'], f32)[:, csl]),
            'Wo': wbf(inputs['Wo']),
            'Wkey': wbf(np.asarray(inputs['Wkey'], f32)[:, ffsl]),
            'Wval': wbf(np.asarray(inputs['Wval'], f32)[ffsl, :]),
            'Wrec': wbf(np.asarray(inputs['Wrec'], f32)[:, csl]),
            'TMK': chvec(inputs['tm_k'], 'tmk'),
            'TMV': chvec(inputs['tm_v'], 'tmv'),
            'TMR': chvec(inputs['tm_r'], 'tmr'),
            'TMG': chvec(inputs['tm_g'], 'tmg'),
            'FMK': chvec(inputs['fm_k'], 'fmk'),
            'FMR': chvec(inputs['fm_r'], 'fmr'),
            'POWR': pair3(pow_r).astype(bf16),
            'POWK': pair3(pow_k).astype(bf16),
            'POWU0': pair2(ulh).astype(f32),
            'POWCT': POW_CT,
            'DLT': pair2(dlh ** L).astype(f32),
            'MASKT': maskT,
            'IDENT': np.ascontiguousarray(ident),
            'PAIRS': pairs,
            'PAIRB': pairb,
            'ONES1': np.ones((1, 128), f32),
            'ONESP': np.ones((128, 1), bf16),
        })
    return in_maps


_LAST_RESULT = {}


def kernel(**inputs):
    global _PROGRAM
    from concourse.bass_utils import run_bass_kernel_spmd
    if _PROGRAM is None:
        _PROGRAM = _build_program()
    in_maps = _host_inputs(inputs)
    trace = bool(int(__import__('os').environ.get('KERNEL_TRACE', '0')))
    res = run_bass_kernel_spmd(_PROGRAM, in_maps, list(range(NCORES)),
                               trace=trace)
    _LAST_RESULT['res'] = res
    out = np.empty((B, T, C), np.float32)
    for core in range(NCORES):
        g, lane = divmod(core, LANES)
        r = res.results[core]
        sl = slice(lane * CHL, (lane + 1) * CHL)
        out[g, :, sl] = (r['o1'] + r['x2out'][sl, :]).T
    return out


# revision 28
# speedup vs baseline: 1.5584x; 1.1042x over previous
"""RWKV-5 block (TimeMix + ChannelMix) on 8 Trainium2 NeuronCores.

Sharding: 2 batch groups x 4-way tensor-parallel (core = 4*g + lane).
TimeMix heads split 8/lane (processed as 4 head-pairs packed in 128
partitions); ChannelMix FF split 2048/lane.

v2 design vs baseline:
- bf16 operands for every matmul (full PE rate incl. small WKV matmuls,
  2x DVE rate, half DMA/SBUF), fp32 PSUM accumulation throughout.
- Channel-major WKV outputs (y = [ch, tok]) eliminate all transposes;
  GroupNorm stats via pair-mask matmuls.
- dv diagonal term computed as u*k*r (decay powers cancel at sigma=tau).
- All intermediates (r,k,kc,v,g,ck,kk,srec,xn,xn2) SBUF-resident; no
  DRAM roundtrips.
- AllGather of att*g chunked 4x and overlapped with the WKV recurrence;
  ReduceScatter of ChannelMix partials chunked 4x and overlapped with
  Wval matmuls.
- LN2 stats accumulated inline with Wo drains; time-shift handled by a
  zero-padded column (no boundary special case).
Host assembles the full [B,T,C] output from per-core row slices.
"""
import sys
import numpy as np

sys.path.insert(0, '/opt/trn_rl_repo')

B, T, C, H, N, FF = 2, 1024, 2048, 32, 64, 8192
EPS = 1e-5
L = 128            # WKV chunk length == token block
NCH = T // L       # 8 chunks
NCORES = 8
LANES = 4
HPL = H // LANES   # 8 heads per lane
NPAIR = HPL // 2   # 4 head pairs
CHL = HPL * N      # 512 att channels per lane
FFL = FF // LANES  # 2048 ff channels per lane
KT = C // 128      # 16 contraction tiles
S = 512            # token free-dim chunk
SP1 = T + 1        # padded token axis (col 0 = zeros for time-shift)
GROUPS = [[0, 1, 2, 3], [4, 5, 6, 7]]

_PROGRAM = None


def _build_program():
    import concourse.bacc as bacc
    import concourse.tile as tile
    from concourse import mybir
    from contextlib import ExitStack

    F32 = mybir.dt.float32
    F32R = mybir.dt.float32r
    BF16 = mybir.dt.bfloat16
    ALU = mybir.AluOpType
    ACT = mybir.ActivationFunctionType

    nc = bacc.Bacc("TRN2", target_bir_lowering=False, debug=False,
                   num_devices=NCORES)

    def din(name, shape, dt=F32):
        return nc.dram_tensor(name, shape, dt, kind="ExternalInput").ap()

    xT = din("xT", [C, T])
    Wr = din("Wr", [C, CHL], BF16); Wk = din("Wk", [C, CHL], BF16)
    Wv = din("Wv", [C, CHL], BF16); Wg = din("Wg", [C, CHL], BF16)
    Wo = din("Wo", [C, C], BF16)
    Wkey = din("Wkey", [C, FFL], BF16); Wval = din("Wval", [FFL, C], BF16)
    Wrec = din("Wrec", [C, CHL], BF16)
    TMK = din("TMK", [128, KT]); TMV = din("TMV", [128, KT])
    TMR = din("TMR", [128, KT]); TMG = din("TMG", [128, KT])
    FMK = din("FMK", [128, KT]); FMR = din("FMR", [128, KT])
    POWR = din("POWR", [128, NPAIR * L], BF16)
    POWK = din("POWK", [128, NPAIR * L], BF16)
    POWU0 = din("POWU0", [128, NPAIR])
    POWCT = din("POWCT", [128, CHL])
    DLT = din("DLT", [128, NPAIR])
    MASKT = din("MASKT", [128, L])
    IDENT = din("IDENT", [128, L], BF16)
    PAIRS = din("PAIRS", [128, 2], BF16)
    PAIRB = din("PAIRB", [2, 128])
    ONES1 = din("ONES1", [1, 128])
    ONESP = din("ONESP", [128, 1], BF16)

    o1 = nc.dram_tensor("o1", [CHL, T], F32, kind="ExternalOutput").ap()
    x2out = nc.dram_tensor("x2out", [C, T], F32, kind="ExternalOutput").ap()

    DEBUG = bool(int(__import__('os').environ.get('KERNEL_DEBUG', '0')))
    dbg_cc = (nc.dram_tensor("dbg_cc", [NCH * CHL, L], BF16,
                             kind="ExternalOutput").ap() if DEBUG else None)
    dbg_ag = (nc.dram_tensor("dbg_ag", [128, KT * NCH * L], BF16,
                             kind="ExternalOutput").ap() if DEBUG else None)
    dbg_rk = (nc.dram_tensor("dbg_rk", [128, 2 * NPAIR * T], BF16,
                             kind="ExternalOutput").ap() if DEBUG else None)
    dbg_vg = (nc.dram_tensor("dbg_vg", [128, NCH * CHL + NPAIR * T], BF16,
                             kind="ExternalOutput").ap() if DEBUG else None)
    dbg_kc = (nc.dram_tensor("dbg_kc", [128, NCH * CHL], BF16,
                             kind="ExternalOutput").ap() if DEBUG else None)
    cc_in = nc.dram_tensor("cc_in", [NCH * CHL, L], BF16).ap()
    ag_out = nc.dram_tensor("ag_out", [16, 2 * CHL * L], BF16).ap()
    rs_in = nc.dram_tensor("rs_in", [4 * C, 256], BF16).ap()
    rs_out = nc.dram_tensor("rs_out", [4 * (C // LANES), 256], BF16).ap()

    with tile.TileContext(nc) as tc, ExitStack() as ctx:
        csts = ctx.enter_context(tc.tile_pool(name="csts", bufs=1))
        big = ctx.enter_context(tc.tile_pool(name="big", bufs=1))
        wbuf = ctx.enter_context(tc.tile_pool(name="wbuf", bufs=1))
        slab = ctx.enter_context(tc.tile_pool(name="slab", bufs=2))
        rot = ctx.enter_context(tc.tile_pool(name="rot", bufs=2))
        rotb = ctx.enter_context(tc.tile_pool(name="rotb", bufs=2))
        wrot = ctx.enter_context(tc.tile_pool(name="wrot", bufs=2))
        state = ctx.enter_context(tc.tile_pool(name="state", bufs=2))
        ps_big = ctx.enter_context(
            tc.tile_pool(name="ps_big", bufs=4, space="PSUM"))
        ps_w1 = ctx.enter_context(
            tc.tile_pool(name="ps_w1", bufs=2, space="PSUM"))
        ps_w2 = ctx.enter_context(
            tc.tile_pool(name="ps_w2", bufs=2, space="PSUM"))

        # ---------------- constants ----------------
        _cst_n = [0]
        def load_const(ap, shape, dt=F32):
            _cst_n[0] += 1
            nm = f"cst{_cst_n[0]}"
            t = csts.tile(shape, dt, name=nm, tag=nm)
            src = ap
            if dt == F32R:
                src = src.bitcast(F32R)
            nc.sync.dma_start(out=t, in_=src)
            return t

        tmK_t = load_const(TMK, [128, KT])
        tmV_t = load_const(TMV, [128, KT])
        tmR_t = load_const(TMR, [128, KT])
        tmG_t = load_const(TMG, [128, KT])
        fmK_t = load_const(FMK, [128, KT])
        fmR_t = load_const(FMR, [128, KT])
        powR_t = load_const(POWR, [128, NPAIR * L], BF16)
        powR_t = powR_t.rearrange("p (m l) -> p m l", m=NPAIR)
        powK_t = load_const(POWK, [128, NPAIR * L], BF16)
        powK_t = powK_t.rearrange("p (m l) -> p m l", m=NPAIR)
        powU0_t = load_const(POWU0, [128, NPAIR])
        powCT_t = load_const(POWCT, [128, CHL])
        dl_t = load_const(DLT, [128, NPAIR])
        maskT_t = load_const(MASKT, [128, L])
        ident_t = load_const(IDENT, [128, L], BF16)
        pairs_t = load_const(PAIRS, [128, 2], BF16)
        pairb_r = load_const(PAIRB, [2, 128], F32R)
        ones1_r = load_const(ONES1, [1, 128], F32R)
        onesp_t = load_const(ONESP, [128, 1], BF16)
        eps_t = csts.tile([1, 1], F32)
        nc.vector.memset(eps_t, EPS)
        geps_t = csts.tile([2, 1], F32)
        nc.vector.memset(geps_t, float(N) * EPS)

        # ---------------- persistent SBUF buffers ----------------
        def bigslot(tag):
            # 32.8KB/partition slots, reused across phases via tags
            return big.tile([128, KT, SP1], BF16, tag=tag, name=tag)

        xn = bigslot("bigA")       # LN1(x), padded; later: ck
        lerpK = bigslot("bigB")    # k-lerp cache; later: xn2 (padded)
        agk = big.tile([128, KT, NCH, L], BF16, tag="bigC", name="agk")
        # agk: AllGather result [p, kt, c, l]; later reused for kk via tag

        rT = wbuf.tile([128, NPAIR, T], BF16, tag="rT", name="rT")
        kT = wbuf.tile([128, NPAIR, T], BF16, tag="kT", name="kT")
        kc = wbuf.tile([128, NCH, CHL], BF16, tag="kc", name="kc")
        vtok = wbuf.tile([128, NCH, CHL], BF16, tag="vtok", name="vtok")
        gch = wbuf.tile([128, NPAIR, T], BF16, tag="gch", name="gch")
        mb_ln1 = wbuf.tile([128, 2, S], BF16, tag="mb1", name="mb1")
        rb_ln1 = wbuf.tile([128, 2, S], BF16, tag="rb1", name="rb1")

        nc.vector.memset(xn[:, :, 0:1], 0.0)
        nc.vector.memset(lerpK[:, :, 0:1], 0.0)

        def load_slab(w_ap, col0):
            # two half-slabs of 256 cols each (8KB/part, bufs=2 rotation)
            halves = []
            for h in range(2):
                t = slab.tile([128, KT, 256], BF16, tag="wsl", name="wsl")
                nc.sync.dma_start(
                    out=t, in_=w_ap[:, col0 + h * 256:
                                    col0 + (h + 1) * 256].rearrange(
                        "(kt p) m -> p kt m", p=128))
                halves.append(t)
            return halves

        def wslice(w_t, kt, mt):
            return w_t[mt // 2][:, kt, (mt % 2) * 128:(mt % 2 + 1) * 128]

        # ================ LN1: load x, stats, normalize ================
        ln1_s = [None, None]
        ln1_q = [None, None]
        for fc in range(2):
            s_t = ps_w1.tile([128, S], F32, tag="w1", name="lnps")
            q_t = ps_w2.tile([128, S], F32, tag="w2", name="lnpq")
            ln1_s[fc] = s_t
            ln1_q[fc] = q_t
            for kt in range(KT):
                xf = rot.tile([128, S], F32, tag="xf", name="xf")
                nc.sync.dma_start(
                    out=xf, in_=xT[kt * 128:(kt + 1) * 128,
                                   fc * S:(fc + 1) * S])
                dst = xn[:, kt, 1 + fc * S:1 + (fc + 1) * S]
                nc.scalar.copy(out=dst, in_=xf)
                sq = rotb.tile([128, S], BF16, tag="sqb", name="sqb")
                nc.vector.tensor_mul(out=sq, in0=dst, in1=dst)
                nc.tensor.matmul(s_t[0:1, :], onesp_t, dst,
                                 start=(kt == 0), stop=(kt == KT - 1))
                nc.tensor.matmul(q_t[0:1, :], onesp_t, sq,
                                 start=(kt == 0), stop=(kt == KT - 1))

        def ln_finalize(s_ts, q_ts, mb, rb):
            for fc in range(2):
                m = rot.tile([1, S], F32R, tag="lnm", name="lnm", bufs=1)
                t1 = rot.tile([1, S], F32, tag="lnt1", name="lnt1", bufs=1)
                t2 = rot.tile([1, S], F32, tag="lnt2", name="lnt2", bufs=1)
                rstd = rot.tile([1, S], F32R, tag="lnr", name="lnr", bufs=1)
                with nc.allow_low_precision("f32r LN vecs for broadcast mm"):
                    nc.vector.tensor_scalar_mul(out=m, in0=s_ts[fc][0:1, :],
                                                scalar1=1.0 / C)
                nc.vector.tensor_scalar_mul(out=t1, in0=q_ts[fc][0:1, :],
                                            scalar1=1.0 / C)
                nc.vector.tensor_mul(out=t2, in0=m.bitcast(F32),
                                     in1=m.bitcast(F32))
                nc.vector.tensor_sub(out=t1, in0=t1, in1=t2)
                nc.scalar.activation(out=t2, in_=t1,
                                     func=ACT.Sqrt, bias=eps_t)
                with nc.allow_low_precision("f32r LN vecs for broadcast mm"):
                    nc.vector.reciprocal(out=rstd, in_=t2)
                for vsl, dst in ((m, mb), (rstd, rb)):
                    ps_b = ps_big.tile([128, S], F32, tag="bm", name="psb")
                    nc.tensor.matmul(ps_b, ones1_r, vsl,
                                     start=True, stop=True)
                    nc.scalar.copy(out=dst[:, fc, :], in_=ps_b)

        ln_finalize(ln1_s, ln1_q, mb_ln1, rb_ln1)
        for fc in range(2):
            for kt in range(KT):
                sl = xn[:, kt, 1 + fc * S:1 + (fc + 1) * S]
                nc.vector.tensor_sub(out=sl, in0=sl, in1=mb_ln1[:, fc, :])
                nc.vector.tensor_mul(out=sl, in0=sl, in1=rb_ln1[:, fc, :])

        # ================ projections ================
        def lerp_into(dst, src, tm_t, kt, c0, cols):
            """dst <- shift-lerp of src tokens [c0, c0+cols) (padded buf)."""
            xl = src[:, kt, 1 + c0:1 + c0 + cols]
            xx = src[:, kt, c0:c0 + cols]
            d = rotb.tile([128, S], BF16, tag="lerpd", name="lerpd")
            nc.vector.tensor_sub(out=d[:, :cols], in0=xl, in1=xx)
            nc.vector.scalar_tensor_tensor(
                out=dst, in0=d[:, :cols], scalar=tm_t[:, kt:kt + 1],
                in1=xx, op0=ALU.mult, op1=ALU.add)

        def lerp_tile(src, tm_t, kt, fc):
            t = rotb.tile([128, S], BF16, tag="lerpt", name="lerpt")
            lerp_into(t, src, tm_t, kt, fc * S, S)
            return t

        def ch_phase(w_t, get_rhs, post):
            for fc in range(2):
                pss = [ps_big.tile([128, S], F32, tag="bm", name="pbm")
                       for _ in range(4)]
                for kt in range(KT):
                    rhs = get_rhs(kt, fc)
                    for mt in range(4):
                        nc.tensor.matmul(
                            pss[mt], wslice(w_t, kt, mt),
                            rhs, start=(kt == 0), stop=(kt == KT - 1))
                for mt in range(4):
                    post(mt, fc, pss[mt])

        # P1: r (channel-major)
        wr_t = load_slab(Wr, 0)
        def post_r(mt, fc, ps):
            nc.scalar.copy(out=rT[:, mt, fc * S:(fc + 1) * S], in_=ps)
        ch_phase(wr_t, lambda kt, fc: lerp_tile(xn, tmR_t, kt, fc), post_r)

        # P2: k channel-major; k-lerp cached into lerpK
        wk_t = load_slab(Wk, 0)
        def krhs(kt, fc):
            dst = lerpK[:, kt, 1 + fc * S:1 + (fc + 1) * S]
            lerp_into(dst, xn, tmK_t, kt, fc * S, S)
            return dst
        def post_k(mt, fc, ps):
            nc.scalar.copy(out=kT[:, mt, fc * S:(fc + 1) * S], in_=ps)
        ch_phase(wk_t, krhs, post_k)

        # P3: k token-major -> kc (uses cached lerpK, same Wk slab)
        def tok_phase(w_t, get_lhs, post):
            # each (q, hf) accumulation chain owns a distinct PSUM bank:
            # interleaved chains within one bank corrupt each other
            # (start=True wipes the whole bank).
            for half in range(2):
                pss = {}
                for q in range(4):
                    pss[(q, 0)] = ps_big.tile([128, 256], F32, tag="bm",
                                              name="pbm")
                for q in range(2):
                    pss[(q, 1)] = ps_w1.tile([128, 256], F32, tag="w1",
                                             name="pw1")
                for q in range(2, 4):
                    pss[(q, 1)] = ps_w2.tile([128, 256], F32, tag="w2",
                                             name="pw2")
                for kt in range(KT):
                    lhs = get_lhs(kt, half)
                    for q in range(4):
                        for hf in range(2):
                            nc.tensor.matmul(
                                pss[(q, hf)],
                                lhs[:, q * 128:(q + 1) * 128],
                                w_t[hf][:, kt, :],
                                start=(kt == 0), stop=(kt == KT - 1))
                for q in range(4):
                    post(half * 4 + q, pss[(q, 0)], pss[(q, 1)])

        def post_kc(tb, ps0, ps1):
            nc.vector.tensor_mul(out=kc[:, tb, 0:256], in0=ps0,
                                 in1=powCT_t[:, 0:256])
            nc.vector.tensor_mul(out=kc[:, tb, 256:512], in0=ps1,
                                 in1=powCT_t[:, 256:512])
        tok_phase(wk_t,
                  lambda kt, half: lerpK[:, kt, 1 + half * S:1 + (half + 1) * S],
                  post_kc)

        # P4: v token-major
        wv_t = load_slab(Wv, 0)
        def post_v(tb, ps0, ps1):
            nc.scalar.copy(out=vtok[:, tb, 0:256], in_=ps0)
            nc.scalar.copy(out=vtok[:, tb, 256:512], in_=ps1)
        tok_phase(wv_t,
                  lambda kt, half: lerp_tile(xn, tmV_t, kt, half),
                  post_v)

        # P5: g channel-major + SiLU
        wg_t = load_slab(Wg, 0)
        def post_g(mt, fc, ps):
            nc.scalar.activation(out=gch[:, mt, fc * S:(fc + 1) * S],
                                 in_=ps, func=ACT.Silu)
        ch_phase(wg_t, lambda kt, fc: lerp_tile(xn, tmG_t, kt, fc), post_g)

        if DEBUG:
            nc.sync.dma_start(out=dbg_rk[:, 0:NPAIR * T],
                              in_=rT.rearrange("p m t -> p (m t)"))
            nc.sync.dma_start(out=dbg_rk[:, NPAIR * T:],
                              in_=kT.rearrange("p m t -> p (m t)"))
            nc.sync.dma_start(out=dbg_vg[:, 0:NCH * CHL],
                              in_=vtok.rearrange("p c n -> p (c n)"))
            nc.sync.dma_start(out=dbg_vg[:, NCH * CHL:],
                              in_=gch.rearrange("p m t -> p (m t)"))
            nc.sync.dma_start(out=dbg_kc,
                              in_=kc.rearrange("p c n -> p (c n)"))

        # ================ WKV recurrence (head pairs) ================
        spairs = {}
        for m in range(NPAIR):
            sp = state.tile([128, 64], BF16, tag=f"St{m}", name="sp")
            nc.vector.memset(sp, 0.0)
            spairs[m] = sp

        for c in range(NCH):
            csl = slice(c * L, (c + 1) * L)
            for m in range(NPAIR):
                rsl = rT[:, m, csl]
                ksl = kT[:, m, csl]
                rdT = wrot.tile([128, L], BF16, tag="rdT", name="rdT")
                nc.vector.tensor_mul(out=rdT, in0=rsl, in1=powR_t[:, m, :])
                kdT = wrot.tile([128, L], BF16, tag="kdT", name="kdT")
                nc.vector.tensor_mul(out=kdT, in0=ksl, in1=powK_t[:, m, :])
                prod = wrot.tile([128, L], BF16, tag="prod", name="prod")
                nc.vector.tensor_mul(out=prod, in0=rsl, in1=ksl)
                nc.vector.tensor_scalar_mul(out=prod, in0=prod,
                                            scalar1=powU0_t[:, m:m + 1])
                # one PSUM bank per accumulation chain (interleaved chains
                # within a bank corrupt each other)
                psA = [ps_w1.tile([128, L], F32, tag="w1", name="psa")
                       for _ in range(2)]
                psDV = [ps_w2.tile([128, 1], F32, tag="w2", name="psdv")
                        for _ in range(2)]
                psY = [ps_big.tile([128, L], F32, tag="bm", name="psy")
                       for _ in range(2)]
                psD = [ps_big.tile([128, 64], F32, tag="bm", name="psd")
                       for _ in range(2)]
                afins = []
                for hh in range(2):
                    b = hh * 64
                    nc.tensor.matmul(psA[hh], kdT[b:b + 64, :],
                                     rdT[b:b + 64, :], start=True, stop=True)
                    nc.tensor.matmul(psDV[hh], prod[b:b + 64, :],
                                     onesp_t[b:b + 64, :],
                                     start=True, stop=True)
                    am = wrot.tile([128, L], BF16, tag="am", name="am")
                    nc.vector.tensor_mul(out=am, in0=psA[hh], in1=maskT_t)
                    dvb = wrot.tile([128, 1], F32, tag="dvb", name="dvb")
                    nc.vector.tensor_copy(out=dvb, in_=psDV[hh])
                    afin = wrot.tile([128, L], BF16, tag="afin", name="afin")
                    nc.vector.scalar_tensor_tensor(
                        out=afin, in0=ident_t, scalar=dvb, in1=am,
                        op0=ALU.mult, op1=ALU.add)
                    afins.append(afin)
                S_pair = spairs[m]
                for hh in range(2):
                    b = hh * 64
                    head = 2 * m + hh
                    hsl = slice(head * 64, (head + 1) * 64)
                    nc.tensor.matmul(psY[hh][b:b + 64, :],
                                     vtok[:, c, hsl], afins[hh],
                                     start=True, stop=False)
                    nc.tensor.matmul(psY[hh][b:b + 64, :],
                                     S_pair[b:b + 64, :], rdT[b:b + 64, :],
                                     start=False, stop=True)
                    nc.tensor.matmul(psD[hh][b:b + 64, :],
                                     kc[:, c, hsl], vtok[:, c, hsl],
                                     start=True, stop=True)
                psDb = wrot.tile([128, 64], BF16, tag="psDb", name="psDb")
                nc.scalar.copy(out=psDb[0:64, :], in_=psD[0][0:64, :])
                nc.scalar.copy(out=psDb[64:128, :], in_=psD[1][64:128, :])
                S_new = state.tile([128, 64], BF16, tag=f"St{m}", name="snew")
                nc.vector.scalar_tensor_tensor(
                    out=S_new, in0=S_pair, scalar=dl_t[:, m:m + 1],
                    in1=psDb, op0=ALU.mult, op1=ALU.add)
                spairs[m] = S_new

                # GroupNorm over each head's 64 channels + gate
                y_sb = wrot.tile([128, L], BF16, tag="ysb", name="ysb")
                nc.scalar.copy(out=y_sb[0:64, :], in_=psY[0][0:64, :])
                nc.scalar.copy(out=y_sb[64:128, :], in_=psY[1][64:128, :])
                ysq = wrot.tile([128, L], BF16, tag="ysq", name="ysq")
                nc.gpsimd.tensor_mul(out=ysq, in0=y_sb, in1=y_sb)
                pstS = ps_w1.tile([2, L], F32, tag="w1", name="pstS")
                pstQ = ps_w1.tile([2, L], F32, tag="w1", name="pstQ")
                nc.tensor.matmul(pstS, pairs_t, y_sb,
                                 start=True, stop=True)
                nc.tensor.matmul(pstQ, pairs_t, ysq,
                                 start=True, stop=True)
                gn = wrot.tile([2, 2 * L], F32R, tag="gn", name="gn", bufs=1)
                m_sb = gn[:, 0:L]
                rstd = gn[:, L:2 * L]
                gn2 = wrot.tile([2, 2 * L], F32, tag="gn2", name="gn2", bufs=1)
                with nc.allow_low_precision("f32r GN vecs for broadcast mm"):
                    nc.vector.tensor_scalar_mul(out=m_sb, in0=pstS,
                                                scalar1=1.0 / N)
                nc.vector.tensor_scalar_mul(out=gn2[:, 0:L],
                                            in0=pstQ,
                                            scalar1=1.0 / N)
                nc.vector.tensor_mul(out=gn2[:, L:2 * L],
                                     in0=m_sb.bitcast(F32),
                                     in1=m_sb.bitcast(F32))
                nc.vector.tensor_sub(out=gn2[:, 0:L],
                                     in0=gn2[:, 0:L], in1=gn2[:, L:2 * L])
                gstd = wrot.tile([2, L], F32, tag="gstd", name="gstd", bufs=1)
                nc.scalar.activation(out=gstd, in_=gn2[:, 0:L],
                                     func=ACT.Sqrt, bias=geps_t)
                with nc.allow_low_precision("f32r GN vecs for broadcast mm"):
                    nc.vector.reciprocal(out=rstd, in_=gstd)
                psM = ps_w2.tile([128, L], F32, tag="w2", name="psM")
                psR = ps_w2.tile([128, L], F32, tag="w2", name="psR")
                nc.tensor.matmul(psM, pairb_r, m_sb, start=True, stop=True)
                nc.tensor.matmul(psR, pairb_r, rstd, start=True, stop=True)
                mbc = wrot.tile([128, L], BF16, tag="mbc", name="mbc")
                nc.scalar.copy(out=mbc, in_=psM)
                rbc = wrot.tile([128, L], BF16, tag="rbc", name="rbc")
                nc.scalar.copy(out=rbc, in_=psR)
                an = wrot.tile([128, L], BF16, tag="an", name="an")
                nc.gpsimd.tensor_sub(out=an, in0=y_sb, in1=mbc)
                nc.vector.tensor_mul(out=an, in0=an, in1=rbc)
                nc.vector.tensor_mul(out=an, in0=an, in1=gch[:, m, csl])
                attg = an
                nc.sync.dma_start(
                    out=cc_in[c * CHL + m * 128:c * CHL + (m + 1) * 128, :],
                    in_=attg)
                if DEBUG:
                    nc.sync.dma_start(
                        out=dbg_cc[c * CHL + m * 128:
                                   c * CHL + (m + 1) * 128, :],
                        in_=attg)
            if c % 2 == 1:
                j = c // 2
                nc.gpsimd.collective_compute(
                    "AllGather", ALU.bypass,
                    ins=[cc_in[(c - 1) * CHL:(c + 1) * CHL, :]],
                    outs=[ag_out[j * 4:(j + 1) * 4, :]],
                    replica_groups=GROUPS)
                half_n = CHL * L
                for r in range(4):
                    for ci in range(2):
                        nc.sync.dma_start(
                            out=agk[:, r * 4:(r + 1) * 4, c - 1 + ci, :],
                            in_=ag_out[j * 4 + r:j * 4 + r + 1,
                                       ci * half_n:(ci + 1) * half_n
                                       ].rearrange(
                                "o (ktl p l) -> p (o ktl) l",
                                ktl=4, p=128))

        # ================ Wo + residual + LN2 stats inline ================
        xn2 = lerpK  # reuse bigB slot (k-lerp cache dead)
        ln2_s = [ps_w1.tile([128, S], F32, tag="w1", name="l2s0"),
                 ps_w1.tile([128, S], F32, tag="w1", name="l2s1")]
        ln2_q = [ps_w2.tile([128, S], F32, tag="w2", name="l2q0"),
                 ps_w2.tile([128, S], F32, tag="w2", name="l2q1")]
        for q in range(4):
            wo_t = load_slab(Wo, q * S)
            for fc in range(2):
                pss = [ps_big.tile([128, S], F32, tag="bm", name="pbm")
                       for _ in range(4)]
                for kt in range(KT):
                    rhs = agk[:, kt, fc * 4:(fc + 1) * 4, :]
                    for mt in range(4):
                        nc.tensor.matmul(
                            pss[mt], wslice(wo_t, kt, mt),
                            rhs, start=(kt == 0), stop=(kt == KT - 1))
                for mt in range(4):
                    gm = q * 4 + mt
                    xres = rot.tile([128, S], F32, tag="xf", name="xres")
                    nc.sync.dma_start(
                        out=xres,
                        in_=xT[gm * 128:(gm + 1) * 128, fc * S:(fc + 1) * S])
                    x2t = rot.tile([128, S], F32, tag="x2t", name="x2t")
                    nc.vector.tensor_add(out=x2t, in0=pss[mt], in1=xres)
                    nc.sync.dma_start(
                        out=x2out[gm * 128:(gm + 1) * 128,
                                  fc * S:(fc + 1) * S],
                        in_=x2t)
                    dst = xn2[:, gm, 1 + fc * S:1 + (fc + 1) * S]
                    nc.scalar.copy(out=dst, in_=x2t)
                    sq2 = rotb.tile([128, S], BF16, tag="sqb", name="sq2")
                    nc.vector.tensor_mul(out=sq2, in0=dst, in1=dst)
                    first = (q == 0 and mt == 0)
                    last = (q == 3 and mt == 3)
                    nc.tensor.matmul(ln2_s[fc][0:1, :], onesp_t, dst,
                                     start=first, stop=last)
                    nc.tensor.matmul(ln2_q[fc][0:1, :], onesp_t, sq2,
                                     start=first, stop=last)

        if DEBUG:
            nc.sync.dma_start(
                out=dbg_ag,
                in_=agk.rearrange("p kt c l -> p (kt c l)"))
        mb_ln2 = wbuf.tile([128, 2, S], BF16, tag="mb1", name="mb2")
        rb_ln2 = wbuf.tile([128, 2, S], BF16, tag="rb1", name="rb2")
        ln_finalize(ln2_s, ln2_q, mb_ln2, rb_ln2)
        for fc in range(2):
            for kt in range(KT):
                sl = xn2[:, kt, 1 + fc * S:1 + (fc + 1) * S]
                nc.vector.tensor_sub(out=sl, in0=sl, in1=mb_ln2[:, fc, :])
                nc.vector.tensor_mul(out=sl, in0=sl, in1=rb_ln2[:, fc, :])

        # ================ ChannelMix ================
        ck = xn  # reuse bigA slot (xn dead)
        srec = wbuf.tile([128, 4, T], BF16, tag="rT", name="srec")
        wrec_t = load_slab(Wrec, 0)
        for fc in range(2):
            pss = [ps_big.tile([128, S], F32, tag="bm", name="pbm")
                   for _ in range(4)]
            for kt in range(KT):
                xl2 = xn2[:, kt, 1 + fc * S:1 + (fc + 1) * S]
                xx2 = xn2[:, kt, fc * S:fc * S + S]
                d2 = rotb.tile([128, S], BF16, tag="lerpd", name="d2")
                nc.vector.tensor_sub(out=d2, in0=xl2, in1=xx2)
                rhs = rotb.tile([128, S], BF16, tag="lerpt", name="crt")
                nc.vector.scalar_tensor_tensor(
                    out=rhs, in0=d2, scalar=fmR_t[:, kt:kt + 1],
                    in1=xx2, op0=ALU.mult, op1=ALU.add)
                nc.vector.scalar_tensor_tensor(
                    out=ck[:, kt, 1 + fc * S:1 + (fc + 1) * S],
                    in0=d2, scalar=fmK_t[:, kt:kt + 1],
                    in1=xx2, op0=ALU.mult, op1=ALU.add)
                for mt in range(4):
                    nc.tensor.matmul(
                        pss[mt], wslice(wrec_t, kt, mt),
                        rhs, start=(kt == 0), stop=(kt == KT - 1))
            for mt in range(4):
                nc.scalar.activation(out=srec[:, mt, fc * S:(fc + 1) * S],
                                     in_=pss[mt], func=ACT.Sigmoid)

        kk = agk.rearrange("p kt c l -> p kt (c l)")  # reuse bigC (ag dead)
        for q in range(4):
            wkey_t = load_slab(Wkey, q * S)
            for fc in range(2):
                pss = [ps_big.tile([128, S], F32, tag="bm", name="pbm")
                       for _ in range(4)]
                for kt in range(KT):
                    for mt in range(4):
                        nc.tensor.matmul(
                            pss[mt], wslice(wkey_t, kt, mt),
                            ck[:, kt, 1 + fc * S:1 + (fc + 1) * S],
                            start=(kt == 0), stop=(kt == KT - 1))
                for mt in range(4):
                    rl = rotb.tile([128, S], BF16, tag="rlb", name="rl")
                    nc.vector.tensor_scalar_max(out=rl, in0=pss[mt],
                                                scalar1=0.0)
                    nc.vector.tensor_mul(
                        out=kk[:, q * 4 + mt, fc * S:(fc + 1) * S],
                        in0=rl, in1=rl)

        for fc in range(2):
            for q in range(4):
                wval_t = load_slab(Wval, q * S)
                pss = [ps_big.tile([128, S], F32, tag="bm", name="pbm")
                       for _ in range(4)]
                for kt in range(KT):
                    for mt in range(4):
                        nc.tensor.matmul(
                            pss[mt], wslice(wval_t, kt, mt),
                            kk[:, kt, fc * S:(fc + 1) * S],
                            start=(kt == 0), stop=(kt == KT - 1))
                for mt in range(4):
                    gm = q * 4 + mt
                    kvt = rotb.tile([128, S], BF16, tag="kvt", name="kvt")
                    nc.scalar.copy(out=kvt, in_=pss[mt])
                    for h in range(2):
                        ch = fc * 2 + h
                        nc.sync.dma_start(
                            out=rs_in[ch * C + gm * 128:
                                      ch * C + (gm + 1) * 128, :],
                            in_=kvt[:, h * 256:(h + 1) * 256])
            for h in range(2):
                ch = fc * 2 + h
                nc.gpsimd.collective_compute(
                    "ReduceScatter", ALU.add,
                    ins=[rs_in[ch * C:(ch + 1) * C, :]],
                    outs=[rs_out[ch * CHL:(ch + 1) * CHL, :]],
                    replica_groups=GROUPS)
            for h in range(2):
                ch = fc * 2 + h
                kvsb = rotb.tile([128, 4, 256], BF16, tag="kvsb",
                                 name="kvsb", bufs=2)
                nc.sync.dma_start(
                    out=kvsb,
                    in_=rs_out[ch * CHL:(ch + 1) * CHL, :].rearrange(
                        "(blk p) t -> p blk t", p=128))
                for blk in range(4):
                    ot = rot.tile([128, 256], F32, tag="ot", name="ot",
                                  bufs=2)
                    nc.vector.tensor_mul(
                        out=ot, in0=srec[:, blk, ch * 256:(ch + 1) * 256],
                        in1=kvsb[:, blk, :])
                    nc.sync.dma_start(
                        out=o1[blk * 128:(blk + 1) * 128,
                               ch * 256:(ch + 1) * 256],
                        in_=ot)

    nc.compile()
    return nc


def _host_inputs(inputs):
    import ml_dtypes
    f32 = np.float32
    bf16 = ml_dtypes.bfloat16
    x = np.asarray(inputs['x'], f32)
    for k in ('ln1_g', 'ln2_g', 'lnx_g'):
        assert np.allclose(np.asarray(inputs[k]), 1.0), f"{k} not identity"
    for k in ('ln1_b', 'ln2_b', 'lnx_b'):
        assert np.allclose(np.asarray(inputs[k]), 0.0), f"{k} not zero"

    dec = np.exp(-np.exp(np.asarray(inputs['time_decay'], np.float64)))
    u = np.asarray(inputs['time_faaaa'], np.float64)
    tau = np.arange(L, dtype=np.float64)

    maskT = np.tril(np.ones((L, L), f32), -1).T.copy()  # [sigma, tau]
    ident = np.eye(L, dtype=bf16)

    def chvec(a, name):
        # [C] -> [128, KT] with channel c = kt*128 + p
        v = np.asarray(a, f32).reshape(C)
        return np.ascontiguousarray(v.reshape(KT, 128).T)

    pairs = np.zeros((128, 2), bf16)
    pairs[0:64, 0] = 1
    pairs[64:128, 1] = 1
    pairb = np.zeros((2, 128), f32)
    pairb[0, 0:64] = 1
    pairb[1, 64:128] = 1

    in_maps = []
    for core in range(NCORES):
        g, lane = divmod(core, LANES)
        hsl = slice(lane * HPL, (lane + 1) * HPL)
        dlh = dec[hsl]            # [HPL, N]
        ulh = u[hsl]

        # pair layout value[p, m] from per-head [HPL, N]:
        # head = 2m + p//64, n = p%64
        def pair2(a):  # [HPL, N] -> [128, NPAIR]
            out = np.empty((128, NPAIR), np.float64)
            for m in range(NPAIR):
                out[0:64, m] = a[2 * m]
                out[64:128, m] = a[2 * m + 1]
            return out

        def pair3(a):  # [HPL, N, L] -> [128, NPAIR*L]
            out = np.empty((128, NPAIR, L), np.float64)
            for m in range(NPAIR):
                out[0:64, m, :] = a[2 * m]
                out[64:128, m, :] = a[2 * m + 1]
            return out.reshape(128, NPAIR * L)

        pow_r = dlh[:, :, None] ** tau[None, None, :]          # [HPL,N,L]
        pow_k = dlh[:, :, None] ** (-(tau[None, None, :] + 1))
        # POWCT: [sigma, ch] = d_ch^(L-1-sigma), ch = hl*64 + n
        pow_c = (dlh[:, :, None] ** (L - 1 - tau[None, None, :]))
        POW_CT = np.ascontiguousarray(
            pow_c.transpose(2, 0, 1).reshape(L, CHL).astype(f32))

        csl = slice(lane * CHL, (lane + 1) * CHL)
        ffsl = slice(lane * FFL, (lane + 1) * FFL)
        wbf = lambda a: np.ascontiguousarray(np.asarray(a, f32)).astype(bf16)
        in_maps.append({
            'xT': np.ascontiguousarray(x[g].T),
            'Wr': wbf(np.asarray(inputs['Wr'], f32)[:, csl]),
            'Wk': wbf(np.asarray(inputs['Wk'], f32)[:, csl]),
            'Wv': wbf(np.asarray(inputs['Wv'], f32)[:, csl]),
            'Wg': wbf(np.asarray(inputs['Wg'], f32)[:, csl]),
            'Wo': wbf(inputs['Wo']),
            'Wkey': wbf(np.asarray(inputs['Wkey'], f32)[:, ffsl]),
            'Wval': wbf(np.asarray(inputs['Wval'], f32)[ffsl, :]),
            'Wrec': wbf(np.asarray(inputs['Wrec'], f32)[:, csl]),
            'TMK': chvec(inputs['tm_k'], 'tmk'),
            'TMV': chvec(inputs['tm_v'], 'tmv'),
            'TMR': chvec(inputs['tm_r'], 'tmr'),
            'TMG': chvec(inputs['tm_g'], 'tmg'),
            'FMK': chvec(inputs['fm_k'], 'fmk'),
            'FMR': chvec(inputs['fm_r'], 'fmr'),
            'POWR': pair3(pow_r).astype(bf16),
            'POWK': pair3(pow_k).astype(bf16),
            'POWU0': pair2(ulh).astype(f32),
            'POWCT': POW_CT,
            'DLT': pair2(dlh ** L).astype(f32),
            'MASKT': maskT,
            'IDENT': np.ascontiguousarray(ident),
            'PAIRS': pairs,
            'PAIRB': pairb,
            'ONES1': np.ones((1, 128), f32),
            'ONESP': np.ones((128, 1), bf16),
        })
    return in_maps


_LAST_RESULT = {}


def kernel(**inputs):
    global _PROGRAM
    from concourse.bass_utils import run_bass_kernel_spmd
    if _PROGRAM is None:
        _PROGRAM = _build_program()
    in_maps = _host_inputs(inputs)
    trace = bool(int(__import__('os').environ.get('KERNEL_TRACE', '0')))
    res = run_bass_kernel_spmd(_PROGRAM, in_maps, list(range(NCORES)),
                               trace=trace)
    _LAST_RESULT['res'] = res
    out = np.empty((B, T, C), np.float32)
    for core in range(NCORES):
        g, lane = divmod(core, LANES)
        r = res.results[core]
        sl = slice(lane * CHL, (lane + 1) * CHL)
        out[g, :, sl] = (r['o1'] + r['x2out'][sl, :]).T
    return out
